# revision 1
# baseline (speedup 1.0000x reference)
"""Tensor-parallel decoder layer (RMSNorm + RoPE causal attention + SwiGLU MLP)
for 8 Trainium2 NeuronCores.

Sharding: q/k/v and gate/up column-sharded (2 heads, 1024 ffn dims per core),
wo/down row-sharded with an fp16 AllReduce after each block. Activations are
kept feature-major (X^T: [D, tokens]) on-chip so every matmul consumes them
without transposes; softmax probabilities are transposed on the PE.

kernel(**inputs) takes the full unsharded inputs and returns the full output.
"""

import math
import numpy as np
from contextlib import ExitStack

import concourse.bass as bass
import concourse.mybir as mybir
import concourse.tile as tile
from concourse import bacc, bass_utils
from concourse.masks import make_identity

f32 = mybir.dt.float32
f16 = mybir.dt.float16

NCORES = 8
P = 128
TCH = 512          # matmul moving free-dim chunk (tokens)
BASE = 10000.0
EPS = 1e-6
EXP_BIAS = -4.0    # constant bias for exp (replaces per-row max subtraction)

FULL_CFG = dict(B=2, T=2048, D=2048, H=16, FF=8192)
TINY_CFG = dict(B=2, T=512, D=1024, H=8, FF=2048)


def _derive(cfg):
    B, T, D, H, FF = cfg["B"], cfg["T"], cfg["D"], cfg["H"], cfg["FF"]
    assert B == 2
    d = dict(cfg)
    d["HD"] = D // H
    assert d["HD"] == P
    d["N"] = B * T            # total tokens
    d["NH"] = H // NCORES     # heads per core
    d["DH"] = d["NH"] * P     # q/k/v width per core
    d["FH"] = FF // NCORES    # ffn width per core
    d["KD"] = D // P          # contraction chunks over D
    d["KF"] = d["FH"] // P    # contraction chunks over ffn shard
    d["CC"] = T // TCH        # token chunks per half (half == batch element)
    d["QT"] = T // P          # query tiles per batch element
    d["NTC"] = d["N"] // TCH  # token chunks total
    d["NAR"] = min(4, d["NTC"])       # all-reduce chunks
    d["GPA"] = d["NTC"] // d["NAR"]   # token chunks per all-reduce chunk
    d["ARCH"] = d["N"] // d["NAR"]    # tokens per all-reduce chunk
    return d


def build_decoder(cfg):
    """Emit the bass program for one core (SPMD across 8)."""
    c = _derive(cfg)
    B, T, D, N = c["B"], c["T"], c["D"], c["N"]
    NH, DH, FH = c["NH"], c["DH"], c["FH"]
    KD, KF, CC, QT = c["KD"], c["KF"], c["CC"], c["QT"]
    NAR, GPA, ARCH = c["NAR"], c["GPA"], c["ARCH"]
    NM = 3 * NH               # q/k/v output tiles per core
    rgroups = [list(range(NCORES))]

    nc = bacc.Bacc("TRN2", target_bir_lowering=False, debug=False,
                   num_devices=NCORES)

    # ---- I/O ----
    xT32 = nc.dram_tensor("xT32", [D, N], f32, kind="ExternalInput")
    xT16 = nc.dram_tensor("xT16", [D, N], f16, kind="ExternalInput")
    cq = nc.dram_tensor("cq", [P, N], f16, kind="ExternalInput")
    sq = nc.dram_tensor("sq", [P, N], f16, kind="ExternalInput")
    ck = nc.dram_tensor("ck", [P, N], f16, kind="ExternalInput")
    sk = nc.dram_tensor("sk", [P, N], f16, kind="ExternalInput")
    maskd = nc.dram_tensor("maskd", [P, P], f32, kind="ExternalInput")
    rotmd = nc.dram_tensor("rotmd", [P, P], f16, kind="ExternalInput")
    wqkv = nc.dram_tensor("wqkv", [D, 3 * DH], f16, kind="ExternalInput")
    wo = nc.dram_tensor("wo", [DH, D], f16, kind="ExternalInput")
    wg = nc.dram_tensor("wg", [D, FH], f16, kind="ExternalInput")
    wu = nc.dram_tensor("wu", [D, FH], f16, kind="ExternalInput")
    wd = nc.dram_tensor("wd", [FH, D], f16, kind="ExternalInput")
    yT = nc.dram_tensor("yT", [D, N], f32, kind="ExternalOutput")

    # collective bounce buffers (per all-reduce chunk, contiguous)
    p1 = [nc.dram_tensor(f"p1_{a}", [D, ARCH], f16) for a in range(NAR)]
    a1 = [nc.dram_tensor(f"a1_{a}", [D, ARCH], f16, addr_space="Shared")
          for a in range(NAR)]
    x1d = nc.dram_tensor("x1d", [D, N], f16)
    p2 = [nc.dram_tensor(f"p2_{a}", [D, ARCH], f16) for a in range(NAR)]
    a2 = [nc.dram_tensor(f"a2_{a}", [D, ARCH], f16, addr_space="Shared")
          for a in range(NAR)]

    with tile.TileContext(nc, pool_alloc_mode="queue") as tc, ExitStack() as ctx:
        constp = ctx.enter_context(tc.tile_pool(name="constp", bufs=1))
        psmall = ctx.enter_context(tc.tile_pool(name="psmall", bufs=1))

        ones_k = constp.tile([P, 1], f16)
        nc.vector.memset(ones_k, 1.0)
        ones_1 = constp.tile([1, P], f16)
        nc.vector.memset(ones_1, 1.0)
        ident = constp.tile([P, P], f16)
        make_identity(nc, ident)
        mask_sb = constp.tile([P, P], f32)
        nc.sync.dma_start(mask_sb, maskd[:, :])
        eps1 = constp.tile([1, 1], f32)
        nc.vector.memset(eps1, EPS)
        ebias = constp.tile([P, 1], f32)
        nc.vector.memset(ebias, EXP_BIAS)
        rot_sb = constp.tile([P, P], f16)
        nc.sync.dma_start(rot_sb, rotmd[:, :])
        wo_sb = constp.tile([P, NH, D], f16)
        nc.sync.dma_start(wo_sb, wo.ap().rearrange("(h p) m -> p h m", p=P))

        # long-lived small tiles
        rsb2 = []     # per token chunk: broadcast 1/rms(x1)  [P, TCH] f16
        for t in range(c["NTC"]):
            r2 = psmall.tile([P, TCH], f16, name=f"rsb2_{t}", tag=f"rsb2_{t}")
            rsb2.append(r2)

        persist = tc.alloc_tile_pool(name="persist", bufs=1)
        # rope'd q,k feature-major per head; v token-major per head; attn out
        qk_f = [persist.tile([P, N], f16, name=f"qkf{m}", tag=f"qkf{m}")
                for m in range(2 * NH)]
        v_sb = [persist.tile([P, N], f16, name=f"vsb{h}", tag=f"vsb{h}")
                for h in range(NH)]
        o_sb = [persist.tile([P, N], f16, name=f"osb{h}", tag=f"osb{h}")
                for h in range(NH)]

        # ================= QKV (+ first RMSNorm) =================
        qp = tc.alloc_tile_pool(name="qkvtrans", bufs=1)
        psq = tc.alloc_tile_pool(name="psumq", bufs=1, space="PSUM")
        for half in range(2):
            toff = half * T
            # stream x^T (f16) for this half; keep all KD chunks resident
            x_sb = []
            for i in range(KD):
                xt = qp.tile([P, T], f16, name=f"xh{i}", tag="xh", bufs=KD)
                nc.sync.dma_start(xt, xT16[i * P:(i + 1) * P, toff:toff + T])
                x_sb.append(xt)
            # sum of squares over D via ones-matmul (row layout [1, TCH]);
            # then rs = 1/sqrt(mean + eps) in row, broadcast and column form
            rsb1 = []
            for cc in range(CC):
                ssq = psq.tile([1, TCH], f32, name="ssq", tag="ssq", bufs=2)
                for i in range(KD):
                    x2 = qp.tile([P, TCH], f16, name="x2", tag="x2", bufs=2)
                    nc.vector.tensor_mul(x2, x_sb[i][:, cc * TCH:(cc + 1) * TCH],
                                         x_sb[i][:, cc * TCH:(cc + 1) * TCH])
                    nc.tensor.matmul(ssq, ones_k, x2,
                                     start=(i == 0), stop=(i == KD - 1))
                srt = qp.tile([1, TCH], f32, name="srt", tag="srt", bufs=2)
                nc.scalar.activation(srt, ssq,
                                     mybir.ActivationFunctionType.Sqrt,
                                     bias=eps1[:, :], scale=1.0 / D)
                rr = qp.tile([1, TCH], f32, name="rr", tag="rr", bufs=2)
                nc.vector.reciprocal(rr, srt)
                rr16 = qp.tile([1, TCH], f16, name="rr16", tag="rr16", bufs=2)
                nc.scalar.copy(rr16, rr)
                rbp = psq.tile([P, TCH], f32, name="rbp", tag="rbp", bufs=1)
                nc.tensor.matmul(rbp, ones_1, rr16, start=True, stop=True)
                rsb = qp.tile([P, TCH], f16, name="rsb", tag="rsb", bufs=CC)
                nc.scalar.copy(rsb, rbp)
                rsb1.append(rsb)
            # tables for this half
            tabs = {}
            for nm, dram in (("cq", cq), ("sq", sq), ("ck", ck), ("sk", sk)):
                tt = qp.tile([P, T], f16, name=nm, tag=f"tab{nm}", bufs=1)
                nc.sync.dma_start(tt, dram[:, toff:toff + T])
                tabs[nm] = tt
            # q/k/v projections, m-tile at a time; token chunks in pairs so a
            # loaded weight tile is reused while only 2 psum banks are held
            for m in range(NM):
                wt = qp.tile([P, KD, P], f16, name="wt", tag="wt", bufs=2)
                nc.sync.dma_start(
                    wt, wqkv.ap()[:, m * P:(m + 1) * P]
                    .rearrange("(k p) m -> p k m", p=P))
                for ccp in range(0, CC, 2):
                    npair = min(2, CC - ccp)
                    pss = [psq.tile([P, TCH], f32, name="qkp", tag="qkp", bufs=2)
                           for _ in range(npair)]
                    for i in range(KD):
                        for u in range(npair):
                            cc = ccp + u
                            nc.tensor.matmul(
                                pss[u], wt[:, i, :],
                                x_sb[i][:, cc * TCH:(cc + 1) * TCH],
                                start=(i == 0), stop=(i == KD - 1))
                    for u in range(npair):
                        cc = ccp + u
                        sl = slice(cc * TCH, (cc + 1) * TCH)
                        gsl = slice(toff + cc * TCH, toff + (cc + 1) * TCH)
                        if m < 2 * NH:
                            # q or k head: scale by rs, apply rope
                            isq = m < NH
                            ct = tabs["cq"] if isq else tabs["ck"]
                            st = tabs["sq"] if isq else tabs["sk"]
                            qh = qp.tile([P, TCH], f16, name="qh", tag="qh",
                                         bufs=2)
                            nc.vector.tensor_tensor(qh, pss[u], rsb1[cc],
                                                    mybir.AluOpType.mult)
                            t1 = qp.tile([P, TCH], f16, name="t1", tag="t1",
                                         bufs=2)
                            nc.vector.tensor_mul(t1, qh, ct[:, sl])
                            rotp = psq.tile([P, TCH], f32, name="rotp",
                                            tag="rotp", bufs=2)
                            nc.tensor.matmul(rotp, rot_sb, qh, start=True,
                                             stop=True)
                            t2 = qp.tile([P, TCH], f16, name="t2", tag="t2",
                                         bufs=2)
                            nc.vector.tensor_tensor(t2, rotp, st[:, sl],
                                                    mybir.AluOpType.mult)
                            nc.vector.tensor_add(qk_f[m][:, gsl], t1, t2)
                        else:
                            # v head: rs-scaled evict, DMA-transpose to
                            # token-major
                            h = m - 2 * NH
                            vtr = qp.tile([P, TCH], f16, name="vtr", tag="vtr",
                                          bufs=2)
                            nc.vector.tensor_tensor(vtr, pss[u], rsb1[cc],
                                                    mybir.AluOpType.mult)
                            for j in range(TCH // P):
                                g = half * (T // P) + cc * (TCH // P) + j
                                nc.sync.dma_start(
                                    v_sb[h][:, g * P:(g + 1) * P],
                                    vtr[:, j * P:(j + 1) * P], transpose=True)
        psq.release()
        qp.release()

        # ================= attention + Wo + AR1 + x1 assembly ============
        ap_ = tc.alloc_tile_pool(name="attntrans", bufs=1)
        psasm = tc.alloc_tile_pool(name="psumasm", bufs=1, space="PSUM")
        psat = tc.alloc_tile_pool(name="psumat", bufs=1, space="PSUM")
        QGRP = TCH // P  # query tiles per Wo token chunk
        for b in range(2):
            boff = b * T
            for qg in range(CC):
                eTb = [ap_.tile([P, TCH], f16, name=f"eTb{kt}", tag="eTb",
                                bufs=QT + 6)
                       for kt in range(qg * QGRP + QGRP)]
                eTb2 = [ap_.tile([P, TCH], f16, name=f"eTc{kt}", tag="eTc",
                                 bufs=QT + 6)
                        for kt in range(qg * QGRP + QGRP)]
                for qt in range(qg * QGRP, (qg + 1) * QGRP):
                    L = (qt + 1) * P
                    nkc = (L + TCH - 1) // TCH
                    for h in range(NH):
                        qv = qk_f[h][:, boff + qt * P: boff + (qt + 1) * P]
                        e_sb = ap_.tile([P, T], f16, name="e", tag="e", bufs=2)
                        rsums = ap_.tile([P, CC], f32, name="rsums", tag="rsums",
                                         bufs=2)
                        for kc in range(nkc):
                            kl = min(TCH, L - kc * TCH)
                            sc = psat.tile([P, TCH], f32, name="sc", tag="sc",
                                           bufs=2)
                            nc.tensor.matmul(
                                sc[:, :kl], qv,
                                qk_f[NH + h][:, boff + kc * TCH: boff + kc * TCH + kl],
                                start=True, stop=True)
                            if kc == qt // QGRP:
                                off = (qt % QGRP) * P
                                nc.vector.tensor_add(sc[:, off:off + P],
                                                     sc[:, off:off + P], mask_sb)
                            nc.scalar.activation(
                                e_sb[:, kc * TCH: kc * TCH + kl], sc[:, :kl],
                                mybir.ActivationFunctionType.Exp,
                                bias=ebias[:, :], scale=1.0,
                                accum_out=rsums[:, kc:kc + 1])
                        rsum = ap_.tile([P, 1], f32, name="rsum", tag="rsum",
                                        bufs=2)
                        nc.vector.tensor_reduce(rsum, rsums[:, 0:nkc],
                                                mybir.AxisListType.X,
                                                mybir.AluOpType.add)
                        rcp = ap_.tile([P, 1], f32, name="rcp", tag="rcp", bufs=2)
                        nc.vector.reciprocal(rcp, rsum)
                        # normalize probabilities in place, then transpose via
                        # DMA into the per-k-tile buffers (h-interleaved cols)
                        nc.vector.tensor_scalar_mul(e_sb[:, :L], e_sb[:, :L],
                                                    rcp)
                        qcol = (qt % QGRP) * P
                        for kt in range(qt + 1):
                            nc.sync.dma_start(
                                eTb[kt][:, qcol:qcol + P] if h == 0 else
                                eTb2[kt][:, qcol:qcol + P],
                                e_sb[:, kt * P:(kt + 1) * P], transpose=True)
                for h in range(NH):
                    buf = eTb if h == 0 else eTb2
                    op_ = psat.tile([P, TCH], f32, name="op", tag="op", bufs=2)
                    for kt in range(qg * QGRP + QGRP):
                        off = max(0, (kt - qg * QGRP)) * P
                        if off >= TCH:
                            break
                        nc.tensor.matmul(
                            op_[:, off:TCH],
                            v_sb[h][:, (b * QT + kt) * P:(b * QT + kt + 1) * P],
                            buf[kt][:, off:TCH],
                            start=(kt == 0), stop=(kt >= qg * QGRP + QGRP - 1))
                    nc.scalar.copy(
                        o_sb[h][:, boff + qg * TCH: boff + (qg + 1) * TCH], op_)
                # Wo partial for this 512-token chunk
                gc = b * CC + qg
                ar = gc // GPA
                colw = (gc % GPA) * TCH
                for mout in range(KD):
                    wop = psasm.tile([P, TCH], f32, name="wop", tag="wop", bufs=1)
                    for h in range(NH):
                        nc.tensor.matmul(
                            wop, wo_sb[:, h, mout * P:(mout + 1) * P],
                            o_sb[h][:, boff + qg * TCH: boff + (qg + 1) * TCH],
                            start=(h == 0), stop=(h == NH - 1))
                    pt = ap_.tile([P, TCH], f16, name="pt", tag="pt", bufs=3)
                    nc.scalar.copy(pt, wop)
                    nc.sync.dma_start(
                        p1[ar][mout * P:(mout + 1) * P, colw:colw + TCH], pt)
                if (gc + 1) % GPA == 0:
                    # all-reduce this chunk, then assemble x1 + second rmsnorm
                    nc.gpsimd.collective_compute(
                        "AllReduce", mybir.AluOpType.add,
                        replica_groups=rgroups,
                        ins=[p1[ar][:, :]], outs=[a1[ar][:, :]])
                    ncc2 = ARCH // TCH
                    for cc2 in range(ncc2):
                        tsl = slice(ar * ARCH + cc2 * TCH,
                                    ar * ARCH + (cc2 + 1) * TCH)
                        csl = slice(cc2 * TCH, (cc2 + 1) * TCH)
                        ssq2 = psasm.tile([1, TCH], f32, name="ssq2", tag="nrm",
                                          bufs=2)
                        for i in range(KD):
                            rsl = slice(i * P, (i + 1) * P)
                            xf = ap_.tile([P, TCH], f16, name="xf", tag="xf",
                                          bufs=2)
                            nc.sync.dma_start(xf, xT16[rsl, tsl])
                            af = ap_.tile([P, TCH], f16, name="af", tag="af",
                                          bufs=2)
                            nc.sync.dma_start(af, a1[ar][rsl, csl])
                            x1t = ap_.tile([P, TCH], f16, name="x1t",
                                           tag="x1t", bufs=3)
                            nc.vector.tensor_add(x1t, xf, af)
                            x2t = ap_.tile([P, TCH], f16, name="x2t", tag="x2t",
                                           bufs=2)
                            nc.vector.tensor_mul(x2t, x1t, x1t)
                            nc.tensor.matmul(ssq2, ones_k, x2t,
                                             start=(i == 0), stop=(i == KD - 1))
                            nc.sync.dma_start(x1d[rsl, tsl], x1t)
                        srt2 = ap_.tile([1, TCH], f32, name="srt2", tag="srt2",
                                        bufs=2)
                        nc.scalar.activation(srt2, ssq2,
                                             mybir.ActivationFunctionType.Sqrt,
                                             bias=eps1[:, :], scale=1.0 / D)
                        rr2 = ap_.tile([1, TCH], f32, name="rr2", tag="rr2",
                                       bufs=2)
                        nc.vector.reciprocal(rr2, srt2)
                        rr216 = ap_.tile([1, TCH], f16, name="rr216", tag="rr216",
                                         bufs=2)
                        nc.scalar.copy(rr216, rr2)
                        rbp3 = psasm.tile([P, TCH], f32, name="rbp3", tag="nrm",
                                          bufs=2)
                        nc.tensor.matmul(rbp3, ones_1, rr216, start=True,
                                         stop=True)
                        nc.scalar.copy(rsb2[ar * ncc2 + cc2], rbp3)
        psat.release()
        ap_.release()
        persist.release()

        # ================= MLP =================
        mp = tc.alloc_tile_pool(name="mlp", bufs=1)
        psm = tc.alloc_tile_pool(name="psumm", bufs=1, space="PSUM")
        wg_sb = mp.tile([P, KD, FH], f16, name="wg_sb", tag="wg_sb")
        nc.sync.dma_start(wg_sb, wg.ap().rearrange("(k p) f -> p k f", p=P))
        wu_sb = mp.tile([P, KD, FH], f16, name="wu_sb", tag="wu_sb")
        nc.sync.dma_start(wu_sb, wu.ap().rearrange("(k p) f -> p k f", p=P))
        wd_sb = mp.tile([P, KF, D], f16, name="wd_sb", tag="wd_sb")
        nc.sync.dma_start(wd_sb, wd.ap().rearrange("(f p) m -> p f m", p=P))
        for cch in range(c["NTC"]):
            ar = cch // GPA
            colw = (cch % GPA) * TCH
            sl = slice(cch * TCH, (cch + 1) * TCH)
            x1c = []
            for i in range(KD):
                xc = mp.tile([P, TCH], f16, name=f"x1c{i}", tag="x1c",
                             bufs=KD + 8)
                nc.sync.dma_start(xc, x1d[i * P:(i + 1) * P, sl])
                x1c.append(xc)
            acs = []
            for fm in range(KF):
                gp = psm.tile([P, TCH], f32, name="gp", tag="gp", bufs=1)
                for i in range(KD):
                    nc.tensor.matmul(gp, wg_sb[:, i, fm * P:(fm + 1) * P],
                                     x1c[i], start=(i == 0), stop=(i == KD - 1))
                up = psm.tile([P, TCH], f32, name="up", tag="up", bufs=1)
                for i in range(KD):
                    nc.tensor.matmul(up, wu_sb[:, i, fm * P:(fm + 1) * P],
                                     x1c[i], start=(i == 0), stop=(i == KD - 1))
                gsc = mp.tile([P, TCH], f16, name="gsc", tag="gsc", bufs=2)
                nc.vector.tensor_tensor(gsc, gp, rsb2[cch], mybir.AluOpType.mult)
                usc = mp.tile([P, TCH], f16, name="usc", tag="usc", bufs=2)
                nc.vector.tensor_tensor(usc, up, rsb2[cch], mybir.AluOpType.mult)
                gss = mp.tile([P, TCH], f16, name="gss", tag="gss", bufs=2)
                nc.scalar.activation(gss, gsc,
                                     mybir.ActivationFunctionType.Silu)
                ac = mp.tile([P, TCH], f16, name="ac", tag="ac", bufs=KF + 2)
                nc.vector.tensor_mul(ac, gss, usc)
                acs.append(ac)
            for mout in range(KD):
                dp = psm.tile([P, TCH], f32, name="dp", tag="dp", bufs=3)
                for fi in range(KF):
                    nc.tensor.matmul(dp, wd_sb[:, fi, mout * P:(mout + 1) * P],
                                     acs[fi], start=(fi == 0), stop=(fi == KF - 1))
                pt2 = mp.tile([P, TCH], f16, name="pt2", tag="pt2", bufs=3)
                nc.scalar.copy(pt2, dp)
                nc.sync.dma_start(
                    p2[ar][mout * P:(mout + 1) * P, colw:colw + TCH], pt2)
            if (cch + 1) % GPA == 0:
                nc.gpsimd.collective_compute(
                    "AllReduce", mybir.AluOpType.add,
                    replica_groups=rgroups,
                    ins=[p2[ar][:, :]], outs=[a2[ar][:, :]])
                # final residual: y = x + attn + mlp
                for i in range(KD):
                    rsl = slice(i * P, (i + 1) * P)
                    for cc2 in range(ARCH // TCH):
                        tsl = slice(ar * ARCH + cc2 * TCH,
                                    ar * ARCH + (cc2 + 1) * TCH)
                        csl = slice(cc2 * TCH, (cc2 + 1) * TCH)
                        yx = mp.tile([P, TCH], f32, name="yx", tag="yx", bufs=2)
                        nc.sync.dma_start(yx, xT32[rsl, tsl])
                        ya = mp.tile([P, TCH], f16, name="ya", tag="ya", bufs=2)
                        nc.sync.dma_start(ya, a1[ar][rsl, csl])
                        yb = mp.tile([P, TCH], f16, name="yb", tag="yb", bufs=2)
                        nc.sync.dma_start(yb, a2[ar][rsl, csl])
                        ys = mp.tile([P, TCH], f32, name="ys", tag="ys", bufs=2)
                        nc.gpsimd.tensor_tensor(ys, yx, ya, mybir.AluOpType.add)
                        nc.gpsimd.tensor_tensor(ys, ys, yb, mybir.AluOpType.add)
                        nc.sync.dma_start(yT[rsl, tsl], ys)
        psm.release()
        psasm.release()
        mp.release()

    nc.compile()
    return nc


# ---------------- host side ----------------

_BUILT = {}


def _get_program(cfg_key, cfg):
    if cfg_key not in _BUILT:
        _BUILT[cfg_key] = build_decoder(cfg)
    return _BUILT[cfg_key]


def _host_prep(cfg, x, position_ids, Wq, Wk, Wv, Wo, Wg, Wu, Wd, g1, g2):
    c = _derive(cfg)
    D, N, DH, FH, HD = c["D"], c["N"], c["DH"], c["FH"], c["HD"]
    xT32 = np.ascontiguousarray(np.asarray(x).reshape(N, D).T).astype(np.float32)
    xT16 = xT32.astype(np.float16)

    pos = np.asarray(position_ids).reshape(-1).astype(np.float32)
    inv_freq = (1.0 / (BASE ** (np.arange(0, HD, 2, dtype=np.float32) / HD)))
    ang = pos[:, None] * inv_freq[None, :]           # [N, HD/2]
    cos_f = np.concatenate([np.cos(ang), np.cos(ang)], axis=1)  # [N, HD]
    sin_f = np.concatenate([np.sin(ang), np.sin(ang)], axis=1)
    s = 1.0 / math.sqrt(HD)
    cqt = np.ascontiguousarray(cos_f.T * s).astype(np.float16)
    sqt = np.ascontiguousarray(sin_f.T * s).astype(np.float16)
    ckt = np.ascontiguousarray(cos_f.T).astype(np.float16)
    skt = np.ascontiguousarray(sin_f.T).astype(np.float16)
    # rotate-half as a permutation matrix: rot(q)[d] = sign(d) * q[(d+64) % 128]
    # lhsT layout for the PE: rotm[k, d] = sign(d) * (k == (d+64) % 128)
    rotm = np.zeros((P, P), np.float16)
    for dd in range(P):
        sgn = -1.0 if dd < P // 2 else 1.0
        rotm[(dd + P // 2) % P, dd] = sgn

    ii, jj = np.indices((P, P))
    maskv = np.where(jj > ii, np.float32(-10000.0), np.float32(0.0))

    g1f = np.asarray(g1, np.float32)[:, None]
    g2f = np.asarray(g2, np.float32)[:, None]
    wqs = (g1f * np.asarray(Wq, np.float32)).astype(np.float16)
    wks = (g1f * np.asarray(Wk, np.float32)).astype(np.float16)
    wvs = (g1f * np.asarray(Wv, np.float32)).astype(np.float16)
    wgs = (g2f * np.asarray(Wg, np.float32)).astype(np.float16)
    wus = (g2f * np.asarray(Wu, np.float32)).astype(np.float16)
    wds = np.asarray(Wd, np.float32).astype(np.float16)
    wos = np.asarray(Wo, np.float32).astype(np.float16)

    in_maps = []
    for i in range(NCORES):
        qs, fs = slice(i * DH, (i + 1) * DH), slice(i * FH, (i + 1) * FH)
        in_maps.append({
            "xT32": xT32, "xT16": xT16,
            "cq": cqt, "sq": sqt, "ck": ckt, "sk": skt,
            "maskd": maskv, "rotmd": rotm,
            "wqkv": np.ascontiguousarray(
                np.concatenate([wqs[:, qs], wks[:, qs], wvs[:, qs]], axis=1)),
            "wo": np.ascontiguousarray(wos[qs, :]),
            "wg": np.ascontiguousarray(wgs[:, fs]),
            "wu": np.ascontiguousarray(wus[:, fs]),
            "wd": np.ascontiguousarray(wds[fs, :]),
        })
    return in_maps


def run(cfg, inputs, **run_kwargs):
    key = tuple(sorted(cfg.items()))
    nc = _get_program(key, cfg)
    in_maps = _host_prep(cfg, **inputs)
    res = bass_utils.run_bass_kernel_spmd(
        nc, in_maps, core_ids=list(range(NCORES)), **run_kwargs)
    yT = res.results[0]["yT"]
    y = np.ascontiguousarray(yT.T).reshape(cfg["B"], cfg["T"], cfg["D"])
    return y.astype(np.float32), res


def kernel(**inputs):
    y, _ = run(FULL_CFG, inputs)
    return y



# revision 5
# speedup vs baseline: 1.3105x; 1.3105x over previous
"""Tensor-parallel decoder layer (RMSNorm + RoPE causal attention + SwiGLU MLP)
for 8 Trainium2 NeuronCores.

Sharding: q/k/v and gate/up column-sharded (2 heads, 1024 ffn dims per core),
wo/down row-sharded with an fp16 AllReduce after each block. Activations are
kept feature-major (X^T: [D, tokens]) on-chip so every matmul consumes them
without transposes; softmax probabilities are transposed on the PE.

kernel(**inputs) takes the full unsharded inputs and returns the full output.
"""

import math
import numpy as np
from contextlib import ExitStack

import concourse.bass as bass
import concourse.mybir as mybir
import concourse.tile as tile
from concourse import bacc, bass_utils
from concourse.masks import make_identity

f32 = mybir.dt.float32
f16 = mybir.dt.float16

NCORES = 8
P = 128
TCH = 512          # matmul moving free-dim chunk (tokens)
BASE = 10000.0
EPS = 1e-6
EXP_BIAS = -4.0    # constant bias for exp (replaces per-row max subtraction)

FULL_CFG = dict(B=2, T=2048, D=2048, H=16, FF=8192)
TINY_CFG = dict(B=2, T=512, D=1024, H=8, FF=2048)


def _derive(cfg):
    B, T, D, H, FF = cfg["B"], cfg["T"], cfg["D"], cfg["H"], cfg["FF"]
    assert B == 2
    d = dict(cfg)
    d["HD"] = D // H
    assert d["HD"] == P
    d["N"] = B * T            # total tokens
    d["NH"] = H // NCORES     # heads per core
    d["DH"] = d["NH"] * P     # q/k/v width per core
    d["FH"] = FF // NCORES    # ffn width per core
    d["KD"] = D // P          # contraction chunks over D
    d["KF"] = d["FH"] // P    # contraction chunks over ffn shard
    d["CC"] = T // TCH        # token chunks per half (half == batch element)
    d["QT"] = T // P          # query tiles per batch element
    d["NTC"] = d["N"] // TCH  # token chunks total
    d["NAR"] = min(4, d["NTC"])       # all-reduce chunks
    d["GPA"] = d["NTC"] // d["NAR"]   # token chunks per all-reduce chunk
    d["ARCH"] = d["N"] // d["NAR"]    # tokens per all-reduce chunk
    return d


def build_decoder(cfg):
    """Emit the bass program for one core (SPMD across 8)."""
    c = _derive(cfg)
    B, T, D, N = c["B"], c["T"], c["D"], c["N"]
    NH, DH, FH = c["NH"], c["DH"], c["FH"]
    KD, KF, CC, QT = c["KD"], c["KF"], c["CC"], c["QT"]
    NAR, GPA, ARCH = c["NAR"], c["GPA"], c["ARCH"]
    NM = 3 * NH               # q/k/v output tiles per core
    rgroups = [list(range(NCORES))]

    nc = bacc.Bacc("TRN2", target_bir_lowering=False, debug=False,
                   num_devices=NCORES)

    # ---- I/O ----
    xT32 = nc.dram_tensor("xT32", [D, N], f32, kind="ExternalInput")
    xT16 = nc.dram_tensor("xT16", [D, N], f16, kind="ExternalInput")
    cq = nc.dram_tensor("cq", [P, N], f16, kind="ExternalInput")
    sq = nc.dram_tensor("sq", [P, N], f16, kind="ExternalInput")
    ck = nc.dram_tensor("ck", [P, N], f16, kind="ExternalInput")
    sk = nc.dram_tensor("sk", [P, N], f16, kind="ExternalInput")
    maskd = nc.dram_tensor("maskd", [P, P], f32, kind="ExternalInput")
    rotmd = nc.dram_tensor("rotmd", [P, P], f16, kind="ExternalInput")
    wqkv = nc.dram_tensor("wqkv", [D, 3 * DH], f16, kind="ExternalInput")
    wo = nc.dram_tensor("wo", [DH, D], f16, kind="ExternalInput")
    wg = nc.dram_tensor("wg", [D, FH], f16, kind="ExternalInput")
    wu = nc.dram_tensor("wu", [D, FH], f16, kind="ExternalInput")
    wd = nc.dram_tensor("wd", [FH, D], f16, kind="ExternalInput")
    yT = nc.dram_tensor("yT", [D, N], f32, kind="ExternalOutput")

    # collective bounce buffers (per all-reduce chunk, contiguous)
    p1 = [nc.dram_tensor(f"p1_{a}", [D, ARCH], f16) for a in range(NAR)]
    a1 = [nc.dram_tensor(f"a1_{a}", [D, ARCH], f16, addr_space="Shared")
          for a in range(NAR)]
    x1d = nc.dram_tensor("x1d", [D, N], f16)
    p2 = [nc.dram_tensor(f"p2_{a}", [D, ARCH], f16) for a in range(NAR)]
    a2 = [nc.dram_tensor(f"a2_{a}", [D, ARCH], f16, addr_space="Shared")
          for a in range(NAR)]

    with tile.TileContext(nc, pool_alloc_mode="queue") as tc, ExitStack() as ctx:
        constp = ctx.enter_context(tc.tile_pool(name="constp", bufs=1))
        psmall = ctx.enter_context(tc.tile_pool(name="psmall", bufs=1))

        ones_k = constp.tile([P, 1], f16)
        nc.vector.memset(ones_k, 1.0)
        ones_1 = constp.tile([1, P], f16)
        nc.vector.memset(ones_1, 1.0)
        ident = constp.tile([P, P], f16)
        make_identity(nc, ident)
        mask_sb = constp.tile([P, P], f32)
        nc.sync.dma_start(mask_sb, maskd[:, :])
        eps1 = constp.tile([1, 1], f32)
        nc.vector.memset(eps1, EPS)
        ebias = constp.tile([P, 1], f32)
        nc.vector.memset(ebias, EXP_BIAS)
        rot_sb = constp.tile([P, P], f16)
        nc.sync.dma_start(rot_sb, rotmd[:, :])
        wo_sb = constp.tile([P, NH, D], f16)
        nc.sync.dma_start(wo_sb, wo.ap().rearrange("(h p) m -> p h m", p=P))

        # long-lived small tiles
        rsb2 = []     # per token chunk: broadcast 1/rms(x1)  [P, TCH] f16
        for t in range(c["NTC"]):
            r2 = psmall.tile([P, TCH], f16, name=f"rsb2_{t}", tag=f"rsb2_{t}")
            rsb2.append(r2)

        persist = tc.alloc_tile_pool(name="persist", bufs=1)
        # rope'd q,k feature-major per head; v token-major per head; attn out
        qk_f = [persist.tile([P, N], f16, name=f"qkf{m}", tag=f"qkf{m}")
                for m in range(2 * NH)]
        v_sb = [persist.tile([P, N], f16, name=f"vsb{h}", tag=f"vsb{h}")
                for h in range(NH)]
        o_sb = [persist.tile([P, N], f16, name=f"osb{h}", tag=f"osb{h}")
                for h in range(NH)]

        # ================= QKV (+ first RMSNorm) =================
        qp = tc.alloc_tile_pool(name="qkvtrans", bufs=1)
        psq = tc.alloc_tile_pool(name="psumq", bufs=1, space="PSUM")
        for half in range(2):
            toff = half * T
            # stream x^T (f16) for this half; keep all KD chunks resident
            x_sb = []
            for i in range(KD):
                xt = qp.tile([P, T], f16, name=f"xh{i}", tag="xh", bufs=KD)
                nc.sync.dma_start(xt, xT16[i * P:(i + 1) * P, toff:toff + T])
                x_sb.append(xt)
            # sum of squares over D via ones-matmul (row layout [1, TCH]);
            # then rs = 1/sqrt(mean + eps) in row, broadcast and column form
            rsb1 = []
            for cc in range(CC):
                ssq = psq.tile([1, TCH], f32, name="ssq", tag="ssq", bufs=2)
                for i in range(KD):
                    x2 = qp.tile([P, TCH], f16, name="x2", tag="x2", bufs=2)
                    nc.vector.tensor_mul(x2, x_sb[i][:, cc * TCH:(cc + 1) * TCH],
                                         x_sb[i][:, cc * TCH:(cc + 1) * TCH])
                    nc.tensor.matmul(ssq, ones_k, x2,
                                     start=(i == 0), stop=(i == KD - 1))
                srt = qp.tile([1, TCH], f32, name="srt", tag="srt", bufs=2)
                nc.scalar.activation(srt, ssq,
                                     mybir.ActivationFunctionType.Sqrt,
                                     bias=eps1[:, :], scale=1.0 / D)
                rr = qp.tile([1, TCH], f32, name="rr", tag="rr", bufs=2)
                nc.vector.reciprocal(rr, srt)
                rr16 = qp.tile([1, TCH], f16, name="rr16", tag="rr16", bufs=2)
                nc.scalar.copy(rr16, rr)
                rbp = psq.tile([P, TCH], f32, name="rbp", tag="rbp", bufs=1)
                nc.tensor.matmul(rbp, ones_1, rr16, start=True, stop=True)
                rsb = qp.tile([P, TCH], f16, name="rsb", tag="rsb", bufs=CC)
                nc.scalar.copy(rsb, rbp)
                rsb1.append(rsb)
            # tables for this half
            tabs = {}
            for nm, dram in (("cq", cq), ("sq", sq), ("ck", ck), ("sk", sk)):
                tt = qp.tile([P, T], f16, name=nm, tag=f"tab{nm}", bufs=1)
                nc.sync.dma_start(tt, dram[:, toff:toff + T])
                tabs[nm] = tt
            # q/k/v projections, m-tile at a time; token chunks in pairs so a
            # loaded weight tile is reused while only 2 psum banks are held
            for m in range(NM):
                wt = qp.tile([P, KD, P], f16, name="wt", tag="wt", bufs=2)
                nc.sync.dma_start(
                    wt, wqkv.ap()[:, m * P:(m + 1) * P]
                    .rearrange("(k p) m -> p k m", p=P))
                for ccp in range(0, CC, 2):
                    npair = min(2, CC - ccp)
                    pss = [psq.tile([P, TCH], f32, name="qkp", tag="qkp", bufs=2)
                           for _ in range(npair)]
                    for i in range(KD):
                        for u in range(npair):
                            cc = ccp + u
                            nc.tensor.matmul(
                                pss[u], wt[:, i, :],
                                x_sb[i][:, cc * TCH:(cc + 1) * TCH],
                                start=(i == 0), stop=(i == KD - 1))
                    for u in range(npair):
                        cc = ccp + u
                        sl = slice(cc * TCH, (cc + 1) * TCH)
                        gsl = slice(toff + cc * TCH, toff + (cc + 1) * TCH)
                        if m < 2 * NH:
                            # q or k head: scale by rs, apply rope
                            isq = m < NH
                            ct = tabs["cq"] if isq else tabs["ck"]
                            st = tabs["sq"] if isq else tabs["sk"]
                            qh = qp.tile([P, TCH], f16, name="qh", tag="qh",
                                         bufs=2)
                            nc.vector.tensor_tensor(qh, pss[u], rsb1[cc],
                                                    mybir.AluOpType.mult)
                            t1 = qp.tile([P, TCH], f16, name="t1", tag="t1",
                                         bufs=2)
                            nc.vector.tensor_mul(t1, qh, ct[:, sl])
                            rotp = psq.tile([P, TCH], f32, name="rotp",
                                            tag="rotp", bufs=2)
                            nc.tensor.matmul(rotp, rot_sb, qh, start=True,
                                             stop=True)
                            t2 = qp.tile([P, TCH], f16, name="t2", tag="t2",
                                         bufs=2)
                            nc.vector.tensor_tensor(t2, rotp, st[:, sl],
                                                    mybir.AluOpType.mult)
                            nc.vector.tensor_add(qk_f[m][:, gsl], t1, t2)
                        else:
                            # v head: rs-scaled evict, DMA-transpose to
                            # token-major
                            h = m - 2 * NH
                            vtr = qp.tile([P, TCH], f16, name="vtr", tag="vtr",
                                          bufs=2)
                            nc.vector.tensor_tensor(vtr, pss[u], rsb1[cc],
                                                    mybir.AluOpType.mult)
                            for j in range(TCH // P):
                                g = half * (T // P) + cc * (TCH // P) + j
                                nc.sync.dma_start(
                                    v_sb[h][:, g * P:(g + 1) * P],
                                    vtr[:, j * P:(j + 1) * P], transpose=True)
        psq.release()
        qp.release()

        # ================= attention + Wo + AR1 + x1 assembly ============
        # scores computed in [k, q] layout: stationary K feature-tile, moving
        # Q chunk -> no probability transposes. softmax denominator via
        # ones-matmul over partitions; 1/rowsum broadcast via ones-matmul and
        # applied to the attention output.
        ap_ = tc.alloc_tile_pool(name="attntrans", bufs=1)
        psasm = tc.alloc_tile_pool(name="psumasm", bufs=1, space="PSUM")
        psat = tc.alloc_tile_pool(name="psumat", bufs=1, space="PSUM")
        QGRP = TCH // P  # 128-token k-tiles per query chunk
        for b in range(2):
            boff = b * T
            for qg in range(CC):
                nkt = (qg + 1) * QGRP
                esb = [[ap_.tile([P, TCH], f16, name=f"e{h}_{kt}", tag="e",
                                 bufs=2 * QT + 4)
                        for kt in range(nkt)] for h in range(NH)]
                # scores + exp (h0 then h1 so exp overlaps next head's MMs)
                for h in range(NH):
                    for kt in range(nkt):
                        jd = kt - qg * QGRP  # >=0: diagonal-group k-tile
                        q0 = max(0, jd) * P
                        sc = psat.tile([P, TCH], f32, name="sc", tag="sc",
                                       bufs=2)
                        nc.tensor.matmul(
                            sc[:, q0:TCH],
                            qk_f[NH + h][:, boff + kt * P: boff + (kt + 1) * P],
                            qk_f[h][:, boff + qg * TCH + q0: boff + (qg + 1) * TCH],
                            start=True, stop=True)
                        if jd >= 0:
                            nc.vector.tensor_add(sc[:, q0:q0 + P],
                                                 sc[:, q0:q0 + P], mask_sb)
                        if q0 > 0:
                            nc.vector.memset(esb[h][kt][:, 0:q0], 0.0)
                        nc.scalar.activation(
                            esb[h][kt][:, q0:TCH], sc[:, q0:TCH],
                            mybir.ActivationFunctionType.Exp,
                            bias=ebias[:, :], scale=1.0)
                # rowsums over k (partitions x tiles) via ones-matmul
                rr16 = []
                for h in range(NH):
                    rsp = psat.tile([1, TCH], f32, name="rs", tag="rs", bufs=1)
                    for kt in range(nkt):
                        nc.tensor.matmul(rsp, ones_k, esb[h][kt],
                                         start=(kt == 0), stop=(kt == nkt - 1))
                    rr = ap_.tile([1, TCH], f32, name="rr", tag="rr", bufs=2)
                    nc.vector.reciprocal(rr, rsp)
                    r16 = ap_.tile([1, TCH], f16, name="r16", tag="r16", bufs=2)
                    nc.scalar.copy(r16, rr)
                    rr16.append(r16)
                for h in range(NH):
                    bcp = psat.tile([P, TCH], f32, name="bc", tag="bc", bufs=1)
                    nc.tensor.matmul(bcp, ones_1, rr16[h], start=True,
                                     stop=True)
                    bc16 = ap_.tile([P, TCH], f16, name="bc16", tag="bc16",
                                    bufs=2)
                    nc.scalar.copy(bc16, bcp)
                    op_ = psat.tile([P, TCH], f32, name="op", tag="op", bufs=1)
                    for kt in range(nkt):
                        nc.tensor.matmul(
                            op_, v_sb[h][:, (b * QT + kt) * P:(b * QT + kt + 1) * P],
                            esb[h][kt], start=(kt == 0), stop=(kt == nkt - 1))
                    nc.vector.tensor_mul(
                        o_sb[h][:, boff + qg * TCH: boff + (qg + 1) * TCH],
                        op_, bc16)
                # Wo partial for this 512-token chunk
                gc = b * CC + qg
                ar = gc // GPA
                colw = (gc % GPA) * TCH
                for mout in range(KD):
                    wop = psasm.tile([P, TCH], f32, name="wop", tag="wop", bufs=1)
                    for h in range(NH):
                        nc.tensor.matmul(
                            wop, wo_sb[:, h, mout * P:(mout + 1) * P],
                            o_sb[h][:, boff + qg * TCH: boff + (qg + 1) * TCH],
                            start=(h == 0), stop=(h == NH - 1))
                    pt = ap_.tile([P, TCH], f16, name="pt", tag="pt", bufs=3)
                    nc.scalar.copy(pt, wop)
                    nc.sync.dma_start(
                        p1[ar][mout * P:(mout + 1) * P, colw:colw + TCH], pt)
                if (gc + 1) % GPA == 0:
                    # all-reduce this chunk, then assemble x1 + second rmsnorm
                    nc.gpsimd.collective_compute(
                        "AllReduce", mybir.AluOpType.add,
                        replica_groups=rgroups,
                        ins=[p1[ar][:, :]], outs=[a1[ar][:, :]])
                    ncc2 = ARCH // TCH
                    for cc2 in range(ncc2):
                        tsl = slice(ar * ARCH + cc2 * TCH,
                                    ar * ARCH + (cc2 + 1) * TCH)
                        csl = slice(cc2 * TCH, (cc2 + 1) * TCH)
                        ssq2 = psasm.tile([1, TCH], f32, name="ssq2", tag="nrm",
                                          bufs=2)
                        for i in range(KD):
                            rsl = slice(i * P, (i + 1) * P)
                            xf = ap_.tile([P, TCH], f16, name="xf", tag="xf",
                                          bufs=2)
                            nc.sync.dma_start(xf, xT16[rsl, tsl])
                            af = ap_.tile([P, TCH], f16, name="af", tag="af",
                                          bufs=2)
                            nc.sync.dma_start(af, a1[ar][rsl, csl])
                            x1t = ap_.tile([P, TCH], f16, name="x1t",
                                           tag="x1t", bufs=3)
                            nc.vector.tensor_add(x1t, xf, af)
                            x2t = ap_.tile([P, TCH], f16, name="x2t", tag="x2t",
                                           bufs=2)
                            nc.vector.tensor_mul(x2t, x1t, x1t)
                            nc.tensor.matmul(ssq2, ones_k, x2t,
                                             start=(i == 0), stop=(i == KD - 1))
                            nc.sync.dma_start(x1d[rsl, tsl], x1t)
                        srt2 = ap_.tile([1, TCH], f32, name="srt2", tag="srt2",
                                        bufs=2)
                        nc.scalar.activation(srt2, ssq2,
                                             mybir.ActivationFunctionType.Sqrt,
                                             bias=eps1[:, :], scale=1.0 / D)
                        rr2 = ap_.tile([1, TCH], f32, name="rr2", tag="rr2",
                                       bufs=2)
                        nc.vector.reciprocal(rr2, srt2)
                        rr216 = ap_.tile([1, TCH], f16, name="rr216", tag="rr216",
                                         bufs=2)
                        nc.scalar.copy(rr216, rr2)
                        rbp3 = psasm.tile([P, TCH], f32, name="rbp3", tag="nrm",
                                          bufs=2)
                        nc.tensor.matmul(rbp3, ones_1, rr216, start=True,
                                         stop=True)
                        nc.scalar.copy(rsb2[ar * ncc2 + cc2], rbp3)
        psat.release()
        ap_.release()
        persist.release()

        # ================= MLP =================
        mp = tc.alloc_tile_pool(name="mlp", bufs=1)
        psm = tc.alloc_tile_pool(name="psumm", bufs=1, space="PSUM")
        wg_sb = mp.tile([P, KD, FH], f16, name="wg_sb", tag="wg_sb")
        nc.sync.dma_start(wg_sb, wg.ap().rearrange("(k p) f -> p k f", p=P))
        wu_sb = mp.tile([P, KD, FH], f16, name="wu_sb", tag="wu_sb")
        nc.sync.dma_start(wu_sb, wu.ap().rearrange("(k p) f -> p k f", p=P))
        wd_sb = mp.tile([P, KF, D], f16, name="wd_sb", tag="wd_sb")
        nc.sync.dma_start(wd_sb, wd.ap().rearrange("(f p) m -> p f m", p=P))
        for cch in range(c["NTC"]):
            ar = cch // GPA
            colw = (cch % GPA) * TCH
            sl = slice(cch * TCH, (cch + 1) * TCH)
            x1c = []
            for i in range(KD):
                xc = mp.tile([P, TCH], f16, name=f"x1c{i}", tag="x1c",
                             bufs=KD + 8)
                nc.sync.dma_start(xc, x1d[i * P:(i + 1) * P, sl])
                x1c.append(xc)
            acs = []
            for fm in range(KF):
                gp = psm.tile([P, TCH], f32, name="gp", tag="gp", bufs=1)
                for i in range(KD):
                    nc.tensor.matmul(gp, wg_sb[:, i, fm * P:(fm + 1) * P],
                                     x1c[i], start=(i == 0), stop=(i == KD - 1))
                up = psm.tile([P, TCH], f32, name="up", tag="up", bufs=1)
                for i in range(KD):
                    nc.tensor.matmul(up, wu_sb[:, i, fm * P:(fm + 1) * P],
                                     x1c[i], start=(i == 0), stop=(i == KD - 1))
                gsc = mp.tile([P, TCH], f16, name="gsc", tag="gsc", bufs=2)
                nc.vector.tensor_tensor(gsc, gp, rsb2[cch], mybir.AluOpType.mult)
                usc = mp.tile([P, TCH], f16, name="usc", tag="usc", bufs=2)
                nc.vector.tensor_tensor(usc, up, rsb2[cch], mybir.AluOpType.mult)
                gss = mp.tile([P, TCH], f16, name="gss", tag="gss", bufs=2)
                nc.scalar.activation(gss, gsc,
                                     mybir.ActivationFunctionType.Silu)
                ac = mp.tile([P, TCH], f16, name="ac", tag="ac", bufs=KF + 2)
                nc.vector.tensor_mul(ac, gss, usc)
                acs.append(ac)
            for mout in range(KD):
                dp = psm.tile([P, TCH], f32, name="dp", tag="dp", bufs=3)
                for fi in range(KF):
                    nc.tensor.matmul(dp, wd_sb[:, fi, mout * P:(mout + 1) * P],
                                     acs[fi], start=(fi == 0), stop=(fi == KF - 1))
                pt2 = mp.tile([P, TCH], f16, name="pt2", tag="pt2", bufs=3)
                nc.scalar.copy(pt2, dp)
                nc.sync.dma_start(
                    p2[ar][mout * P:(mout + 1) * P, colw:colw + TCH], pt2)
            if (cch + 1) % GPA == 0:
                nc.gpsimd.collective_compute(
                    "AllReduce", mybir.AluOpType.add,
                    replica_groups=rgroups,
                    ins=[p2[ar][:, :]], outs=[a2[ar][:, :]])
                # final residual: y = x + attn + mlp
                for i in range(KD):
                    rsl = slice(i * P, (i + 1) * P)
                    for cc2 in range(ARCH // TCH):
                        tsl = slice(ar * ARCH + cc2 * TCH,
                                    ar * ARCH + (cc2 + 1) * TCH)
                        csl = slice(cc2 * TCH, (cc2 + 1) * TCH)
                        yx = mp.tile([P, TCH], f32, name="yx", tag="yx", bufs=2)
                        nc.sync.dma_start(yx, xT32[rsl, tsl])
                        ya = mp.tile([P, TCH], f16, name="ya", tag="ya", bufs=2)
                        nc.sync.dma_start(ya, a1[ar][rsl, csl])
                        yb = mp.tile([P, TCH], f16, name="yb", tag="yb", bufs=2)
                        nc.sync.dma_start(yb, a2[ar][rsl, csl])
                        ys = mp.tile([P, TCH], f32, name="ys", tag="ys", bufs=2)
                        nc.gpsimd.tensor_tensor(ys, yx, ya, mybir.AluOpType.add)
                        nc.gpsimd.tensor_tensor(ys, ys, yb, mybir.AluOpType.add)
                        nc.sync.dma_start(yT[rsl, tsl], ys)
        psm.release()
        psasm.release()
        mp.release()

    nc.compile()
    return nc


# ---------------- host side ----------------

_BUILT = {}


def _get_program(cfg_key, cfg):
    if cfg_key not in _BUILT:
        _BUILT[cfg_key] = build_decoder(cfg)
    return _BUILT[cfg_key]


def _host_prep(cfg, x, position_ids, Wq, Wk, Wv, Wo, Wg, Wu, Wd, g1, g2):
    c = _derive(cfg)
    D, N, DH, FH, HD = c["D"], c["N"], c["DH"], c["FH"], c["HD"]
    xT32 = np.ascontiguousarray(np.asarray(x).reshape(N, D).T).astype(np.float32)
    xT16 = xT32.astype(np.float16)

    pos = np.asarray(position_ids).reshape(-1).astype(np.float32)
    inv_freq = (1.0 / (BASE ** (np.arange(0, HD, 2, dtype=np.float32) / HD)))
    ang = pos[:, None] * inv_freq[None, :]           # [N, HD/2]
    cos_f = np.concatenate([np.cos(ang), np.cos(ang)], axis=1)  # [N, HD]
    sin_f = np.concatenate([np.sin(ang), np.sin(ang)], axis=1)
    s = 1.0 / math.sqrt(HD)
    cqt = np.ascontiguousarray(cos_f.T * s).astype(np.float16)
    sqt = np.ascontiguousarray(sin_f.T * s).astype(np.float16)
    ckt = np.ascontiguousarray(cos_f.T).astype(np.float16)
    skt = np.ascontiguousarray(sin_f.T).astype(np.float16)
    # rotate-half as a permutation matrix: rot(q)[d] = sign(d) * q[(d+64) % 128]
    # lhsT layout for the PE: rotm[k, d] = sign(d) * (k == (d+64) % 128)
    rotm = np.zeros((P, P), np.float16)
    for dd in range(P):
        sgn = -1.0 if dd < P // 2 else 1.0
        rotm[(dd + P // 2) % P, dd] = sgn

    ii, jj = np.indices((P, P))
    maskv = np.where(ii > jj, np.float32(-10000.0), np.float32(0.0))

    g1f = np.asarray(g1, np.float32)[:, None]
    g2f = np.asarray(g2, np.float32)[:, None]
    wqs = (g1f * np.asarray(Wq, np.float32)).astype(np.float16)
    wks = (g1f * np.asarray(Wk, np.float32)).astype(np.float16)
    wvs = (g1f * np.asarray(Wv, np.float32)).astype(np.float16)
    wgs = (g2f * np.asarray(Wg, np.float32)).astype(np.float16)
    wus = (g2f * np.asarray(Wu, np.float32)).astype(np.float16)
    wds = np.asarray(Wd, np.float32).astype(np.float16)
    wos = np.asarray(Wo, np.float32).astype(np.float16)

    in_maps = []
    for i in range(NCORES):
        qs, fs = slice(i * DH, (i + 1) * DH), slice(i * FH, (i + 1) * FH)
        in_maps.append({
            "xT32": xT32, "xT16": xT16,
            "cq": cqt, "sq": sqt, "ck": ckt, "sk": skt,
            "maskd": maskv, "rotmd": rotm,
            "wqkv": np.ascontiguousarray(
                np.concatenate([wqs[:, qs], wks[:, qs], wvs[:, qs]], axis=1)),
            "wo": np.ascontiguousarray(wos[qs, :]),
            "wg": np.ascontiguousarray(wgs[:, fs]),
            "wu": np.ascontiguousarray(wus[:, fs]),
            "wd": np.ascontiguousarray(wds[fs, :]),
        })
    return in_maps


def run(cfg, inputs, **run_kwargs):
    key = tuple(sorted(cfg.items()))
    nc = _get_program(key, cfg)
    in_maps = _host_prep(cfg, **inputs)
    res = bass_utils.run_bass_kernel_spmd(
        nc, in_maps, core_ids=list(range(NCORES)), **run_kwargs)
    yT = res.results[0]["yT"]
    y = np.ascontiguousarray(yT.T).reshape(cfg["B"], cfg["T"], cfg["D"])
    return y.astype(np.float32), res


def kernel(**inputs):
    y, _ = run(FULL_CFG, inputs)
    return y



# revision 15
# speedup vs baseline: 1.9680x; 1.5017x over previous
"""Decoder layer (RMSNorm + RoPE causal attention + SwiGLU MLP) on 8 TRN2
NeuronCores.

Attention is tensor-parallel over heads (2 heads/core); scores are computed
in [k, q] layout (stationary K feature-tile, moving Q chunk) so no
probability transposes are needed. Wo partials are produced token-major and
ReduceScattered so each core ends up owning 4 x 128 tokens; the MLP then
runs fully locally per core on those 512 tokens with full-size (replicated,
streamed) gate/up/down weights. No AllReduce, no full-activation bounce.

kernel(**inputs) takes the full unsharded inputs and returns the full output.
"""

import math
import numpy as np
from contextlib import ExitStack

import concourse.bass as bass
import concourse.mybir as mybir
import concourse.tile as tile
from concourse import bacc, bass_utils
from concourse.masks import make_identity

f32 = mybir.dt.float32
f16 = mybir.dt.float16

NCORES = 8
P = 128
TCH = 512          # matmul moving free-dim chunk (tokens)
BASE = 10000.0
EPS = 1e-6
EXP_BIAS = -4.0    # constant bias for exp (replaces per-row max subtraction)

FULL_CFG = dict(B=2, T=2048, D=2048, H=16, FF=8192)


def _derive(cfg):
    B, T, D, H, FF = cfg["B"], cfg["T"], cfg["D"], cfg["H"], cfg["FF"]
    assert B == 2
    d = dict(cfg)
    d["HD"] = D // H
    assert d["HD"] == P
    d["N"] = B * T            # total tokens
    d["NH"] = H // NCORES     # heads per core
    d["DH"] = d["NH"] * P     # q/k/v width per core
    d["KD"] = D // P          # contraction chunks over D
    d["FFT"] = FF // P        # ff tiles (full, replicated MLP)
    d["CC"] = T // TCH        # token chunks per batch element
    d["QT"] = T // P          # 128-token tiles per batch element
    d["NTC"] = d["N"] // TCH  # token chunks total
    d["NAR"] = 4              # reduce-scatter groups
    d["GPA"] = d["NTC"] // d["NAR"]   # token chunks per RS group
    d["GT"] = d["N"] // d["NAR"]      # tokens per RS group
    d["FC"] = D // TCH        # feature chunks of the model dim
    assert d["GT"] // NCORES == P     # own tokens per group == P
    return d


def build_decoder(cfg):
    """Emit the bass program for one core (SPMD across 8)."""
    c = _derive(cfg)
    B, T, D, N = c["B"], c["T"], c["D"], c["N"]
    NH, DH = c["NH"], c["DH"]
    KD, CC, QT, FFT = c["KD"], c["CC"], c["QT"], c["FFT"]
    NAR, GPA, GT, FC = c["NAR"], c["GPA"], c["GT"], c["FC"]
    NM = 3 * NH               # q/k/v output tiles per core
    rgroups = [list(range(NCORES))]

    nc = bacc.Bacc("TRN2", target_bir_lowering=False, debug=False,
                   num_devices=NCORES)

    # ---- I/O ----
    xT16 = nc.dram_tensor("xT16", [D, N], f16, kind="ExternalInput")
    xown = nc.dram_tensor("xown", [NAR * P, D], f16, kind="ExternalInput")
    cq = nc.dram_tensor("cq", [P, N], f16, kind="ExternalInput")
    sq = nc.dram_tensor("sq", [P, N], f16, kind="ExternalInput")
    ck = nc.dram_tensor("ck", [P, N], f16, kind="ExternalInput")
    sk = nc.dram_tensor("sk", [P, N], f16, kind="ExternalInput")
    maskd = nc.dram_tensor("maskd", [P, P], f32, kind="ExternalInput")
    rotmd = nc.dram_tensor("rotmd", [P, P], f16, kind="ExternalInput")
    wqkv = nc.dram_tensor("wqkv", [D, 3 * DH], f16, kind="ExternalInput")
    wo = nc.dram_tensor("wo", [DH, D], f16, kind="ExternalInput")
    wgd = nc.dram_tensor("wgd", [FFT * P, KD * P], f16, kind="ExternalInput")
    wud = nc.dram_tensor("wud", [FFT * P, KD * P], f16, kind="ExternalInput")
    wdd = nc.dram_tensor("wdd", [cfg["FF"], D], f16, kind="ExternalInput")
    yOut = nc.dram_tensor("yOut", [NAR * P, D], f32, kind="ExternalOutput")

    # collective buffers (token-major Wo partials per RS group)
    p1g = [nc.dram_tensor(f"p1g_{g}", [GT, D], f16) for g in range(NAR)]
    a1g = [nc.dram_tensor(f"a1g_{g}", [P, D], f16) for g in range(NAR)]

    with tile.TileContext(nc, pool_alloc_mode="queue") as tc, ExitStack() as ctx:
        constp = ctx.enter_context(tc.tile_pool(name="constp", bufs=1))

        ones_k = constp.tile([P, 1], f16)
        nc.vector.memset(ones_k, 1.0)
        ones_1 = constp.tile([1, P], f16)
        nc.vector.memset(ones_1, 1.0)
        ones_pp = constp.tile([P, P], f16)
        nc.vector.memset(ones_pp, 1.0)
        ident = constp.tile([P, P], f16)
        make_identity(nc, ident)
        mask_sb = constp.tile([P, P], f32)
        nc.sync.dma_start(mask_sb, maskd[:, :])
        eps1 = constp.tile([1, 1], f32)
        nc.vector.memset(eps1, EPS)
        epsP = constp.tile([P, 1], f32)
        nc.vector.memset(epsP, EPS)
        ebias = constp.tile([P, 1], f32)
        nc.vector.memset(ebias, EXP_BIAS)
        rot_sb = constp.tile([P, P], f16)
        nc.sync.dma_start(rot_sb, rotmd[:, :])
        wo_sb = constp.tile([P, NH, D], f16)
        nc.sync.dma_start(wo_sb, wo.ap().rearrange("(h p) m -> p h m", p=P))

        persist = tc.alloc_tile_pool(name="persist", bufs=1)
        # rope'd q,k feature-major per head; v token-major per head; attn out
        qk_f = [persist.tile([P, N], f16, name=f"qkf{m}", tag=f"qkf{m}")
                for m in range(2 * NH)]
        v_sb = [persist.tile([P, N], f16, name=f"vsb{h}", tag=f"vsb{h}")
                for h in range(NH)]
        o_sb = [persist.tile([P, N], f16, name=f"osb{h}", tag=f"osb{h}")
                for h in range(NH)]

        # ================= QKV (+ first RMSNorm) =================
        qp = tc.alloc_tile_pool(name="qkvtrans", bufs=1)
        psq = tc.alloc_tile_pool(name="psumq", bufs=1, space="PSUM")
        for half in range(2):
            toff = half * T
            # stream x^T (f16) for this half; keep all KD chunks resident
            x_sb = []
            for i in range(KD):
                xt = qp.tile([P, T], f16, name=f"xh{i}", tag="xh", bufs=KD)
                nc.sync.dma_start(xt, xT16[i * P:(i + 1) * P, toff:toff + T])
                x_sb.append(xt)
            # sum of squares over D via ones-matmul (row layout [1, TCH]);
            # then rs = 1/sqrt(mean + eps) in row, broadcast and column form
            rsb1 = []
            for cc in range(CC):
                ssq = psq.tile([1, TCH], f32, name="ssq", tag="ssq", bufs=2)
                for i in range(KD):
                    x2 = qp.tile([P, TCH], f16, name="x2", tag="x2", bufs=2)
                    nc.vector.tensor_mul(x2, x_sb[i][:, cc * TCH:(cc + 1) * TCH],
                                         x_sb[i][:, cc * TCH:(cc + 1) * TCH])
                    nc.tensor.matmul(ssq, ones_k, x2,
                                     start=(i == 0), stop=(i == KD - 1))
                rr16 = qp.tile([1, TCH], f16, name="rr16", tag="rr16", bufs=2)
                nc.scalar.activation(rr16, ssq,
                                     mybir.ActivationFunctionType.Rsqrt,
                                     bias=eps1[:, :], scale=1.0 / D)
                rbp = psq.tile([P, TCH], f32, name="rbp", tag="rbp", bufs=1)
                nc.tensor.matmul(rbp, ones_1, rr16, start=True, stop=True)
                rsb = qp.tile([P, TCH], f16, name="rsb", tag="rsb", bufs=CC)
                nc.scalar.copy(rsb, rbp)
                rsb1.append(rsb)
            # tables for this half
            tabs = {}
            for nm, dram in (("cq", cq), ("sq", sq), ("ck", ck), ("sk", sk)):
                tt = qp.tile([P, T], f16, name=nm, tag=f"tab{nm}", bufs=1)
                nc.sync.dma_start(tt, dram[:, toff:toff + T])
                tabs[nm] = tt
            # q/k/v projections, m-tile at a time; token chunks in pairs so a
            # loaded weight tile is reused while only 2 psum banks are held
            for m in range(NM):
                wt = qp.tile([P, KD, P], f16, name="wt", tag="wt", bufs=2)
                nc.sync.dma_start(
                    wt, wqkv.ap()[:, m * P:(m + 1) * P]
                    .rearrange("(k p) m -> p k m", p=P))
                for ccp in range(0, CC, 2):
                    npair = min(2, CC - ccp)
                    pss = [psq.tile([P, TCH], f32, name="qkp", tag="qkp", bufs=2)
                           for _ in range(npair)]
                    for i in range(KD):
                        for u in range(npair):
                            cc = ccp + u
                            nc.tensor.matmul(
                                pss[u], wt[:, i, :],
                                x_sb[i][:, cc * TCH:(cc + 1) * TCH],
                                start=(i == 0), stop=(i == KD - 1))
                    for u in range(npair):
                        cc = ccp + u
                        sl = slice(cc * TCH, (cc + 1) * TCH)
                        gsl = slice(toff + cc * TCH, toff + (cc + 1) * TCH)
                        if m < 2 * NH:
                            # q or k head: scale by rs, apply rope
                            isq = m < NH
                            ct = tabs["cq"] if isq else tabs["ck"]
                            st = tabs["sq"] if isq else tabs["sk"]
                            qh = qp.tile([P, TCH], f16, name="qh", tag="qh",
                                         bufs=2)
                            nc.vector.tensor_tensor(qh, pss[u], rsb1[cc],
                                                    mybir.AluOpType.mult)
                            t1 = qp.tile([P, TCH], f16, name="t1", tag="t1",
                                         bufs=2)
                            nc.vector.tensor_mul(t1, qh, ct[:, sl])
                            rotp = psq.tile([P, TCH], f32, name="rotp",
                                            tag="rotp", bufs=2)
                            nc.tensor.matmul(rotp, rot_sb, qh, start=True,
                                             stop=True)
                            t2 = qp.tile([P, TCH], f16, name="t2", tag="t2",
                                         bufs=2)
                            nc.vector.tensor_tensor(t2, rotp, st[:, sl],
                                                    mybir.AluOpType.mult)
                            nc.vector.tensor_add(qk_f[m][:, gsl], t1, t2)
                        else:
                            # v head: rs-scaled evict, DMA-transpose to
                            # token-major
                            h = m - 2 * NH
                            vtr = qp.tile([P, TCH], f16, name="vtr", tag="vtr",
                                          bufs=2)
                            nc.vector.tensor_tensor(vtr, pss[u], rsb1[cc],
                                                    mybir.AluOpType.mult)
                            for j in range(TCH // P):
                                g = half * (T // P) + cc * (TCH // P) + j
                                nc.sync.dma_start(
                                    v_sb[h][:, g * P:(g + 1) * P],
                                    vtr[:, j * P:(j + 1) * P], transpose=True)
        psq.release()
        qp.release()

        # ========== attention ([k,q] layout) + token-major Wo + RS ==========
        ap_ = tc.alloc_tile_pool(name="attntrans", bufs=1)
        psat = tc.alloc_tile_pool(name="psumat", bufs=1, space="PSUM")
        QGRP = TCH // P  # 128-token k-tiles per query chunk
        for b in range(2):
            boff = b * T
            for qg in range(CC):
                nkt = (qg + 1) * QGRP
                esb = [[ap_.tile([P, TCH], f16, name=f"e{h}_{kt}", tag="e",
                                 bufs=2 * QT + 4)
                        for kt in range(nkt)] for h in range(NH)]
                # scores + exp (h0 then h1 so exp overlaps next head's MMs)
                for h in range(NH):
                    for kt in range(nkt):
                        jd = kt - qg * QGRP  # >=0: diagonal-group k-tile
                        q0 = max(0, jd) * P
                        sc = psat.tile([P, TCH], f32, name="sc", tag="sc",
                                       bufs=2)
                        nc.tensor.matmul(
                            sc[:, q0:TCH],
                            qk_f[NH + h][:, boff + kt * P: boff + (kt + 1) * P],
                            qk_f[h][:, boff + qg * TCH + q0: boff + (qg + 1) * TCH],
                            start=True, stop=True)
                        if jd >= 0:
                            nc.vector.tensor_add(sc[:, q0:q0 + P],
                                                 sc[:, q0:q0 + P], mask_sb)
                        if q0 > 0:
                            nc.vector.memset(esb[h][kt][:, 0:q0], 0.0)
                        nc.scalar.activation(
                            esb[h][kt][:, q0:TCH], sc[:, q0:TCH],
                            mybir.ActivationFunctionType.Exp,
                            bias=ebias[:, :], scale=1.0)
                # rowsum (broadcast across partitions via all-ones stationary)
                rcp16 = []
                for h in range(NH):
                    rbp = psat.tile([P, TCH], f32, name="rb", tag="rb", bufs=2)
                    for kt in range(nkt):
                        nc.tensor.matmul(rbp, ones_pp, esb[h][kt],
                                         start=(kt == 0), stop=(kt == nkt - 1))
                    r16 = ap_.tile([P, TCH], f16, name="r16", tag="r16", bufs=2)
                    nc.scalar.activation(r16, rbp,
                                         mybir.ActivationFunctionType.Reciprocal)
                    rcp16.append(r16)
                for h in range(NH):
                    op_ = psat.tile([P, TCH], f32, name="op", tag="op", bufs=2)
                    for kt in range(nkt):
                        nc.tensor.matmul(
                            op_, v_sb[h][:, (b * QT + kt) * P:(b * QT + kt + 1) * P],
                            esb[h][kt], start=(kt == 0), stop=(kt == nkt - 1))
                    nc.vector.tensor_mul(
                        o_sb[h][:, boff + qg * TCH: boff + (qg + 1) * TCH],
                        op_, rcp16[h])
                # token-major Wo partial for this 512-token chunk
                gc = b * CC + qg
                g = gc // GPA
                coff = (gc % GPA) * TCH
                for tt in range(QGRP):
                    for fc in range(FC):
                        wop = psat.tile([P, TCH], f32, name="wop", tag="wop",
                                        bufs=2)
                        for h in range(NH):
                            nc.tensor.matmul(
                                wop,
                                o_sb[h][:, boff + qg * TCH + tt * P:
                                        boff + qg * TCH + (tt + 1) * P],
                                wo_sb[:, h, fc * TCH:(fc + 1) * TCH],
                                start=(h == 0), stop=(h == NH - 1))
                        pt = ap_.tile([P, TCH], f16, name="pt", tag="pt",
                                      bufs=4)
                        if (tt + fc) % 2 == 0:
                            nc.scalar.copy(pt, wop)
                        else:
                            nc.vector.tensor_copy(pt, wop)
                        nc.sync.dma_start(
                            p1g[g][coff + tt * P: coff + (tt + 1) * P,
                                   fc * TCH:(fc + 1) * TCH], pt)
                if (gc + 1) % GPA == 0:
                    nc.gpsimd.collective_compute(
                        "ReduceScatter", mybir.AluOpType.add,
                        replica_groups=rgroups,
                        ins=[p1g[g][:, :]], outs=[a1g[g][:, :]])
        psat.release()
        ap_.release()
        persist.release()

        # ========== local MLP on own 4x128 tokens (full FF, streamed) =======
        mp = tc.alloc_tile_pool(name="mlp", bufs=1)
        pstp = tc.alloc_tile_pool(name="psumtp", bufs=1, space="PSUM")
        x1g = [mp.tile([P, D], f16, name=f"x1g{g}", tag=f"x1g{g}")
               for g in range(NAR)]
        x1nT = [mp.tile([P, NAR * P], f16, name=f"xnt{i}", tag=f"xnt{i}")
                for i in range(KD)]
        for g in range(NAR):
            ag = mp.tile([P, D], f16, name="ag", tag="ag", bufs=2)
            nc.sync.dma_start(ag, a1g[g][:, :])
            xg = mp.tile([P, D], f16, name="xg", tag="xg", bufs=2)
            nc.sync.dma_start(xg, xown[g * P:(g + 1) * P, :])
            nc.vector.tensor_add(x1g[g], xg, ag)
            sqv = mp.tile([P, D], f16, name="sqv", tag="sqv", bufs=2)
            ssq = mp.tile([P, 1], f32, name="ssq", tag="ssq", bufs=2)
            nc.scalar.activation(sqv, x1g[g],
                                 mybir.ActivationFunctionType.Square,
                                 accum_out=ssq)
            rsg = mp.tile([P, 1], f32, name="rsg", tag="rsg", bufs=2)
            nc.scalar.activation(rsg, ssq, mybir.ActivationFunctionType.Rsqrt,
                                 bias=epsP[:, :], scale=1.0 / D)
            x1n = mp.tile([P, D], f16, name="x1n", tag="x1n", bufs=2)
            nc.vector.tensor_scalar_mul(x1n, x1g[g], rsg)
            # transpose to feature-major via PE
            for i in range(KD):
                tp = pstp.tile([P, P], f16, name="tp", tag="tp", bufs=2)
                nc.tensor.transpose(tp, x1n[:, i * P:(i + 1) * P], ident)
                nc.scalar.copy(x1nT[i][:, g * P:(g + 1) * P], tp)
        pstp.release()
        psm = tc.alloc_tile_pool(name="psumm", bufs=1, space="PSUM")
        # gate/up
        acs = []
        for m in range(FFT):
            wgt = mp.tile([P, KD * P], f16, name="wgt", tag="wgt", bufs=3)
            nc.sync.dma_start(wgt, wgd[m * P:(m + 1) * P, :])
            wut = mp.tile([P, KD * P], f16, name="wut", tag="wut", bufs=3)
            nc.sync.dma_start(wut, wud[m * P:(m + 1) * P, :])
            gp = psm.tile([P, NAR * P], f32, name="gp", tag="gp", bufs=2)
            for i in range(KD):
                nc.tensor.matmul(gp, wgt[:, i * P:(i + 1) * P], x1nT[i],
                                 start=(i == 0), stop=(i == KD - 1))
            up = psm.tile([P, NAR * P], f32, name="upp", tag="upp", bufs=2)
            for i in range(KD):
                nc.tensor.matmul(up, wut[:, i * P:(i + 1) * P], x1nT[i],
                                 start=(i == 0), stop=(i == KD - 1))
            gs = mp.tile([P, NAR * P], f16, name="gs", tag="gs", bufs=2)
            nc.scalar.activation(gs, gp, mybir.ActivationFunctionType.Silu)
            ac = mp.tile([P, NAR * P], f16, name="ac", tag="ac", bufs=FFT + 1)
            nc.vector.tensor_mul(ac, gs, up)
            acs.append(ac)
        # down + final residual
        for fc in range(FC):
            dps = [psm.tile([P, TCH], f32, name=f"dp{tt}", tag=f"dp{tt}",
                            bufs=1) for tt in range(NAR)]
            for m in range(FFT):
                wdt = mp.tile([P, TCH], f16, name="wdt", tag="wdt", bufs=4)
                nc.sync.dma_start(wdt, wdd[m * P:(m + 1) * P,
                                           fc * TCH:(fc + 1) * TCH])
                for tt in range(NAR):
                    nc.tensor.matmul(dps[tt], acs[m][:, tt * P:(tt + 1) * P],
                                     wdt, start=(m == 0), stop=(m == FFT - 1))
            for tt in range(NAR):
                yt = mp.tile([P, TCH], f32, name="yt", tag="yt", bufs=3)
                nc.vector.tensor_add(yt, dps[tt],
                                     x1g[tt][:, fc * TCH:(fc + 1) * TCH])
                nc.sync.dma_start(
                    yOut[tt * P:(tt + 1) * P, fc * TCH:(fc + 1) * TCH], yt)
        psm.release()
        mp.release()

    nc.compile()
    return nc


# ---------------- host side ----------------

_BUILT = {}


def _get_program(cfg_key, cfg):
    if cfg_key not in _BUILT:
        _BUILT[cfg_key] = build_decoder(cfg)
    return _BUILT[cfg_key]


def _host_prep(cfg, x, position_ids, Wq, Wk, Wv, Wo, Wg, Wu, Wd, g1, g2):
    c = _derive(cfg)
    D, N, DH, HD = c["D"], c["N"], c["DH"], c["HD"]
    KD, FFT, NAR, GT = c["KD"], c["FFT"], c["NAR"], c["GT"]
    FF = cfg["FF"]
    xN = np.asarray(x).reshape(N, D).astype(np.float32)
    xN16 = xN.astype(np.float16)
    xT16 = np.ascontiguousarray(xN16.T)

    pos = np.asarray(position_ids).reshape(-1).astype(np.float32)
    inv_freq = (1.0 / (BASE ** (np.arange(0, HD, 2, dtype=np.float32) / HD)))
    ang = pos[:, None] * inv_freq[None, :]           # [N, HD/2]
    cos_f = np.concatenate([np.cos(ang), np.cos(ang)], axis=1)  # [N, HD]
    sin_f = np.concatenate([np.sin(ang), np.sin(ang)], axis=1)
    s = 1.0 / math.sqrt(HD)
    cqt = np.ascontiguousarray(cos_f.T * s).astype(np.float16)
    sqt = np.ascontiguousarray(sin_f.T * s).astype(np.float16)
    ckt = np.ascontiguousarray(cos_f.T).astype(np.float16)
    skt = np.ascontiguousarray(sin_f.T).astype(np.float16)
    # rotate-half as a permutation matrix: rot(q)[d] = sign(d) * q[(d+64) % 128]
    # lhsT layout for the PE: rotm[k, d] = sign(d) * (k == (d+64) % 128)
    rotm = np.zeros((P, P), np.float16)
    for dd in range(P):
        sgn = -1.0 if dd < P // 2 else 1.0
        rotm[(dd + P // 2) % P, dd] = sgn

    # [k, q] score layout: invalid where k > q
    ii, jj = np.indices((P, P))
    maskv = np.where(ii > jj, np.float32(-10000.0), np.float32(0.0))

    g1f = np.asarray(g1, np.float32)[:, None]
    g2f = np.asarray(g2, np.float32)[:, None]
    wqs = (g1f * np.asarray(Wq, np.float32)).astype(np.float16)
    wks = (g1f * np.asarray(Wk, np.float32)).astype(np.float16)
    wvs = (g1f * np.asarray(Wv, np.float32)).astype(np.float16)
    wgs = (g2f * np.asarray(Wg, np.float32)).astype(np.float16)
    wus = (g2f * np.asarray(Wu, np.float32)).astype(np.float16)
    wds = np.asarray(Wd, np.float32).astype(np.float16)
    wos = np.asarray(Wo, np.float32).astype(np.float16)

    # swizzle gate/up so each 128-wide ff tile's [P, KD*P] lhsT block is a
    # contiguous DMA: wgd[m*P+p, k*P+j] = wgs[k*P+p, m*P+j]
    def _swz(w):
        return np.ascontiguousarray(
            w.reshape(KD, P, FFT, P).transpose(2, 1, 0, 3)
            .reshape(FFT * P, KD * P))

    wgd = _swz(wgs)
    wud = _swz(wus)

    in_maps = []
    for i in range(NCORES):
        qs = slice(i * DH, (i + 1) * DH)
        xo = np.concatenate(
            [xN16[g * GT + i * P: g * GT + (i + 1) * P] for g in range(NAR)],
            axis=0)
        in_maps.append({
            "xT16": xT16, "xown": np.ascontiguousarray(xo),
            "cq": cqt, "sq": sqt, "ck": ckt, "sk": skt,
            "maskd": maskv, "rotmd": rotm,
            "wqkv": np.ascontiguousarray(
                np.concatenate([wqs[:, qs], wks[:, qs], wvs[:, qs]], axis=1)),
            "wo": np.ascontiguousarray(wos[qs, :]),
            "wgd": wgd, "wud": wud, "wdd": wds,
        })
    return in_maps


def run(cfg, inputs, **run_kwargs):
    key = tuple(sorted(cfg.items()))
    nc = _get_program(key, cfg)
    c = _derive(cfg)
    in_maps = _host_prep(cfg, **inputs)
    res = bass_utils.run_bass_kernel_spmd(
        nc, in_maps, core_ids=list(range(NCORES)), **run_kwargs)
    N, D, NAR, GT = c["N"], c["D"], c["NAR"], c["GT"]
    y = np.empty((N, D), np.float32)
    for i in range(NCORES):
        yo = np.asarray(res.results[i]["yOut"])
        for g in range(NAR):
            y[g * GT + i * P: g * GT + (i + 1) * P] = yo[g * P:(g + 1) * P]
    return y.reshape(cfg["B"], cfg["T"], cfg["D"]), res


def kernel(**inputs):
    y, _ = run(FULL_CFG, inputs)
    return y


# revision 31
# speedup vs baseline: 2.0970x; 1.0655x over previous
"""Decoder layer (RMSNorm + RoPE causal attention + SwiGLU MLP) on 8 TRN2
NeuronCores.

Attention is tensor-parallel over heads (2 heads/core); scores are computed
in [k, q] layout (stationary K feature-tile, moving Q chunk) so no
probability transposes are needed. Wo partials are produced token-major and
ReduceScattered so each core ends up owning 4 x 128 tokens; the MLP then
runs fully locally per core on those 512 tokens with full-size (replicated,
streamed) gate/up/down weights. No AllReduce, no full-activation bounce.

kernel(**inputs) takes the full unsharded inputs and returns the full output.
"""

import math
import numpy as np
from contextlib import ExitStack

import concourse.bass as bass
import concourse.mybir as mybir
import concourse.tile as tile
from concourse import bacc, bass_utils
from concourse.masks import make_identity

f32 = mybir.dt.float32
f16 = mybir.dt.float16

NCORES = 8
P = 128
TCH = 512          # matmul moving free-dim chunk (tokens)
BASE = 10000.0
EPS = 1e-6
EXP_BIAS = -4.0    # constant bias for exp (replaces per-row max subtraction)

FULL_CFG = dict(B=2, T=2048, D=2048, H=16, FF=8192)


def _derive(cfg):
    B, T, D, H, FF = cfg["B"], cfg["T"], cfg["D"], cfg["H"], cfg["FF"]
    assert B == 2
    d = dict(cfg)
    d["HD"] = D // H
    assert d["HD"] == P
    d["N"] = B * T            # total tokens
    d["NH"] = H // NCORES     # heads per core
    d["DH"] = d["NH"] * P     # q/k/v width per core
    d["KD"] = D // P          # contraction chunks over D
    d["FFT"] = FF // P        # ff tiles (full, replicated MLP)
    d["CC"] = T // TCH        # token chunks per batch element
    d["QT"] = T // P          # 128-token tiles per batch element
    d["NTC"] = d["N"] // TCH  # token chunks total
    d["NAR"] = 4              # reduce-scatter groups
    d["GPA"] = d["NTC"] // d["NAR"]   # token chunks per RS group
    d["GT"] = d["N"] // d["NAR"]      # tokens per RS group
    d["FC"] = D // TCH        # feature chunks of the model dim
    assert d["GT"] // NCORES == P     # own tokens per group == P
    return d


def build_decoder(cfg):
    """Emit the bass program for one core (SPMD across 8)."""
    c = _derive(cfg)
    B, T, D, N = c["B"], c["T"], c["D"], c["N"]
    NH, DH = c["NH"], c["DH"]
    KD, CC, QT, FFT = c["KD"], c["CC"], c["QT"], c["FFT"]
    NAR, GPA, GT, FC = c["NAR"], c["GPA"], c["GT"], c["FC"]
    NM = 3 * NH               # q/k/v output tiles per core
    rgroups = [list(range(NCORES))]

    nc = bacc.Bacc("TRN2", target_bir_lowering=False, debug=False,
                   num_devices=NCORES)

    # ---- I/O ----
    xT16 = nc.dram_tensor("xT16", [D, N], f16, kind="ExternalInput")
    xown = nc.dram_tensor("xown", [NAR * P, D], f16, kind="ExternalInput")
    cq = nc.dram_tensor("cq", [P, N], f16, kind="ExternalInput")
    sq = nc.dram_tensor("sq", [P, N], f16, kind="ExternalInput")
    ck = nc.dram_tensor("ck", [P, N], f16, kind="ExternalInput")
    sk = nc.dram_tensor("sk", [P, N], f16, kind="ExternalInput")
    maskd = nc.dram_tensor("maskd", [P, P], f32, kind="ExternalInput")
    rotmd = nc.dram_tensor("rotmd", [P, P], f16, kind="ExternalInput")
    wqkv = nc.dram_tensor("wqkv", [3 * DH, KD * P], f16, kind="ExternalInput")
    wo = nc.dram_tensor("wo", [DH, D], f16, kind="ExternalInput")
    wgd = nc.dram_tensor("wgd", [FFT * P, KD * P], f16, kind="ExternalInput")
    wud = nc.dram_tensor("wud", [FFT * P, KD * P], f16, kind="ExternalInput")
    wdd = nc.dram_tensor("wdd", [cfg["FF"], D], f16, kind="ExternalInput")
    yOut = nc.dram_tensor("yOut", [NAR * P, D], f32, kind="ExternalOutput")

    # collective buffers (token-major Wo partials per RS group)
    p1g = [nc.dram_tensor(f"p1g_{g}", [GT, D], f16) for g in range(NAR)]
    a1g = [nc.dram_tensor(f"a1g_{g}", [P, D], f16) for g in range(NAR)]

    with tile.TileContext(nc, pool_alloc_mode="queue") as tc, ExitStack() as ctx:
        constp = ctx.enter_context(tc.tile_pool(name="constp", bufs=1))

        ones_k = constp.tile([P, 1], f16)
        nc.vector.memset(ones_k, 1.0)
        ones_1 = constp.tile([1, P], f16)
        nc.vector.memset(ones_1, 1.0)
        ones_pp = constp.tile([P, P], f16)
        nc.vector.memset(ones_pp, 1.0)
        ident = constp.tile([P, P], f16)
        make_identity(nc, ident)
        mask_sb = constp.tile([P, P], f32)
        nc.sync.dma_start(mask_sb, maskd[:, :])
        eps1 = constp.tile([1, 1], f32)
        nc.vector.memset(eps1, EPS)
        epsP = constp.tile([P, 1], f32)
        nc.vector.memset(epsP, EPS)
        ebias = constp.tile([P, 1], f32)
        nc.vector.memset(ebias, EXP_BIAS)
        rot_sb = constp.tile([P, P], f16)
        nc.sync.dma_start(rot_sb, rotmd[:, :])
        wo_sb = constp.tile([P, NH, D], f16)
        nc.sync.dma_start(wo_sb, wo.ap().rearrange("(h p) m -> p h m", p=P))

        persist = tc.alloc_tile_pool(name="persist", bufs=1)
        # rope'd q,k feature-major per head; v token-major per head; attn out
        qk_f = [persist.tile([P, N], f16, name=f"qkf{m}", tag=f"qkf{m}")
                for m in range(2 * NH)]
        v_sb = [persist.tile([P, N], f16, name=f"vsb{h}", tag=f"vsb{h}")
                for h in range(NH)]
        o_sb = [persist.tile([P, N], f16, name=f"osb{h}", tag=f"osb{h}")
                for h in range(NH)]

        # ================= QKV (+ first RMSNorm) =================
        qp = tc.alloc_tile_pool(name="qkvtrans", bufs=1)
        psq = tc.alloc_tile_pool(name="psumq", bufs=1, space="PSUM")
        for half in range(2):
            toff = half * T
            # stream x^T (f16) for this half; keep all KD chunks resident
            x_sb = []
            for i in range(KD):
                xt = qp.tile([P, T], f16, name=f"xh{i}", tag="xh", bufs=KD + 2)
                nc.sync.dma_start(xt, xT16[i * P:(i + 1) * P, toff:toff + T])
                x_sb.append(xt)
            # sum of squares over D via ones-matmul (row layout [1, TCH]);
            # then rs = 1/sqrt(mean + eps) in row, broadcast and column form
            rsb1 = []
            for cc in range(CC):
                rbq = psq.tile([P, TCH], f32, name="rbq", tag="rbq", bufs=2)
                for i in range(KD):
                    x2 = qp.tile([P, TCH], f16, name="x2", tag="x2", bufs=2)
                    nc.vector.tensor_mul(x2, x_sb[i][:, cc * TCH:(cc + 1) * TCH],
                                         x_sb[i][:, cc * TCH:(cc + 1) * TCH])
                    nc.tensor.matmul(rbq, ones_pp, x2,
                                     start=(i == 0), stop=(i == KD - 1))
                srt = qp.tile([P, TCH], f32, name="srt", tag="srt", bufs=2)
                nc.scalar.activation(srt, rbq,
                                     mybir.ActivationFunctionType.Sqrt,
                                     bias=epsP[:, :], scale=1.0 / D)
                rsb = qp.tile([P, TCH], f16, name="rsb", tag="rsb", bufs=CC)
                with nc.allow_low_precision(reason="rmsnorm 1/rms"):
                    nc.vector.reciprocal(rsb, srt)
                rsb1.append(rsb)
            # tables for this half
            tabs = {}
            for nm, dram in (("cq", cq), ("sq", sq), ("ck", ck), ("sk", sk)):
                tt = qp.tile([P, T], f16, name=nm, tag=f"tab{nm}", bufs=1)
                nc.sync.dma_start(tt, dram[:, toff:toff + T])
                tabs[nm] = tt
            # q/k/v projections, m-tile at a time; token chunks in pairs so a
            # loaded weight tile is reused while only 2 psum banks are held
            for m in range(NM):
                wt = qp.tile([P, KD * P], f16, name="wt", tag="wt", bufs=2)
                nc.sync.dma_start(wt, wqkv[m * P:(m + 1) * P, :])
                for ccp in range(0, CC, 2):
                    npair = min(2, CC - ccp)
                    pss = [psq.tile([P, TCH], f32, name="qkp", tag="qkp", bufs=2)
                           for _ in range(npair)]
                    for i in range(KD):
                        for u in range(npair):
                            cc = ccp + u
                            nc.tensor.matmul(
                                pss[u], wt[:, i * P:(i + 1) * P],
                                x_sb[i][:, cc * TCH:(cc + 1) * TCH],
                                start=(i == 0), stop=(i == KD - 1))
                    for u in range(npair):
                        cc = ccp + u
                        sl = slice(cc * TCH, (cc + 1) * TCH)
                        gsl = slice(toff + cc * TCH, toff + (cc + 1) * TCH)
                        if m < 2 * NH:
                            # q or k head: scale by rs, apply rope
                            isq = m < NH
                            ct = tabs["cq"] if isq else tabs["ck"]
                            st = tabs["sq"] if isq else tabs["sk"]
                            qh = qp.tile([P, TCH], f16, name="qh", tag="qh",
                                         bufs=2)
                            nc.vector.tensor_tensor(qh, pss[u], rsb1[cc],
                                                    mybir.AluOpType.mult)
                            t1 = qp.tile([P, TCH], f16, name="t1", tag="t1",
                                         bufs=2)
                            nc.vector.tensor_mul(t1, qh, ct[:, sl])
                            rotp = psq.tile([P, TCH], f32, name="rotp",
                                            tag="rotp", bufs=2)
                            nc.tensor.matmul(rotp, rot_sb, qh, start=True,
                                             stop=True)
                            t2 = qp.tile([P, TCH], f16, name="t2", tag="t2",
                                         bufs=2)
                            nc.vector.tensor_tensor(t2, rotp, st[:, sl],
                                                    mybir.AluOpType.mult)
                            nc.vector.tensor_add(qk_f[m][:, gsl], t1, t2)
                        else:
                            # v head: rs-scaled evict, DMA-transpose to
                            # token-major
                            h = m - 2 * NH
                            vtr = qp.tile([P, TCH], f16, name="vtr", tag="vtr",
                                          bufs=2)
                            nc.vector.tensor_tensor(vtr, pss[u], rsb1[cc],
                                                    mybir.AluOpType.mult)
                            for j in range(TCH // P):
                                g = half * (T // P) + cc * (TCH // P) + j
                                nc.sync.dma_start(
                                    v_sb[h][:, g * P:(g + 1) * P],
                                    vtr[:, j * P:(j + 1) * P], transpose=True)
        psq.release()
        qp.release()

        # ========== attention ([k,q] layout) + token-major Wo + RS ==========
        ap_ = tc.alloc_tile_pool(name="attntrans", bufs=1)
        psat = tc.alloc_tile_pool(name="psumat", bufs=1, space="PSUM")
        mprep = tc.alloc_tile_pool(name="mprep", bufs=1)
        x1g = [mprep.tile([P, D], f16, name=f"x1g{g}", tag=f"x1g{g}")
               for g in range(NAR)]
        x1nT = [mprep.tile([P, NAR * P], f16, name=f"xnt{i}", tag=f"xnt{i}")
                for i in range(KD)]

        def mlp_prep(g):
            """x1 = x + attn for own tokens of RS group g, rmsnorm, and
            PE-transpose into the feature-major x1nT tiles."""
            ag = mprep.tile([P, D], f16, name="ag", tag="ag", bufs=2)
            nc.sync.dma_start(ag, a1g[g][:, :])
            xg = mprep.tile([P, D], f16, name="xg", tag="xg", bufs=2)
            nc.sync.dma_start(xg, xown[g * P:(g + 1) * P, :])
            nc.vector.tensor_add(x1g[g], xg, ag)
            sqv = mprep.tile([P, D], f16, name="sqv", tag="sqv", bufs=2)
            ssq = mprep.tile([P, 1], f32, name="ssq", tag="ssq", bufs=2)
            nc.scalar.activation(sqv, x1g[g],
                                 mybir.ActivationFunctionType.Square,
                                 accum_out=ssq)
            srt = mprep.tile([P, 1], f32, name="srt", tag="srt", bufs=2)
            nc.scalar.activation(srt, ssq, mybir.ActivationFunctionType.Sqrt,
                                 bias=epsP[:, :], scale=1.0 / D)
            rsg = mprep.tile([P, 1], f32, name="rsg", tag="rsg", bufs=2)
            nc.vector.reciprocal(rsg, srt)
            x1n = mprep.tile([P, D], f16, name="x1n", tag="x1n", bufs=2)
            nc.vector.tensor_scalar_mul(x1n, x1g[g], rsg)
            for i in range(KD):
                tp = psat.tile([P, P], f16, name="tp", tag="tp", bufs=1)
                nc.tensor.transpose(tp, x1n[:, i * P:(i + 1) * P], ident)
                nc.scalar.copy(x1nT[i][:, g * P:(g + 1) * P], tp)

        QGRP = TCH // P  # 128-token k-tiles per query chunk
        for b in range(2):
            boff = b * T
            for qg in range(CC):
                nkt = (qg + 1) * QGRP
                esb = [[ap_.tile([P, TCH], f16, name=f"e{h}_{kt}", tag="e",
                                 bufs=2 * QT + 4)
                        for kt in range(nkt)] for h in range(NH)]
                # scores + exp (h0 then h1 so exp overlaps next head's MMs)
                for h in range(NH):
                    for kt in range(nkt):
                        jd = kt - qg * QGRP  # >=0: diagonal-group k-tile
                        q0 = max(0, jd) * P
                        sc = psat.tile([P, TCH], f32, name="sc", tag="sc",
                                       bufs=2)
                        nc.tensor.matmul(
                            sc[:, q0:TCH],
                            qk_f[NH + h][:, boff + kt * P: boff + (kt + 1) * P],
                            qk_f[h][:, boff + qg * TCH + q0: boff + (qg + 1) * TCH],
                            start=True, stop=True)
                        if jd >= 0:
                            nc.vector.tensor_add(sc[:, q0:q0 + P],
                                                 sc[:, q0:q0 + P], mask_sb)
                        if q0 > 0:
                            nc.vector.memset(esb[h][kt][:, 0:q0], 0.0)
                        nc.scalar.activation(
                            esb[h][kt][:, q0:TCH], sc[:, q0:TCH],
                            mybir.ActivationFunctionType.Exp,
                            bias=ebias[:, :], scale=1.0)
                # per-head: rowsum (all-ones broadcast), 1/sum on DVE
                # overlapping the AV matmuls, then normalize the output
                for h in range(NH):
                    rbp = psat.tile([P, TCH], f32, name="rb", tag="rb", bufs=2)
                    for kt in range(nkt):
                        nc.tensor.matmul(rbp, ones_pp, esb[h][kt],
                                         start=(kt == 0), stop=(kt == nkt - 1))
                    r16 = ap_.tile([P, TCH], f16, name="r16", tag="r16", bufs=2)
                    with nc.allow_low_precision(reason="softmax 1/rowsum"):
                        nc.vector.reciprocal(r16, rbp)
                    op_ = psat.tile([P, TCH], f32, name="op", tag="op", bufs=1)
                    for kt in range(nkt):
                        nc.tensor.matmul(
                            op_, v_sb[h][:, (b * QT + kt) * P:(b * QT + kt + 1) * P],
                            esb[h][kt], start=(kt == 0), stop=(kt == nkt - 1))
                    nc.vector.tensor_mul(
                        o_sb[h][:, boff + qg * TCH: boff + (qg + 1) * TCH],
                        op_, r16)
                # token-major Wo partial for this 512-token chunk
                gc = b * CC + qg
                g = gc // GPA
                coff = (gc % GPA) * TCH
                for tt in range(QGRP):
                    for fc in range(FC):
                        wop = psat.tile([P, TCH], f32, name="wop", tag="wop",
                                        bufs=2)
                        for h in range(NH):
                            nc.tensor.matmul(
                                wop,
                                o_sb[h][:, boff + qg * TCH + tt * P:
                                        boff + qg * TCH + (tt + 1) * P],
                                wo_sb[:, h, fc * TCH:(fc + 1) * TCH],
                                start=(h == 0), stop=(h == NH - 1))
                        pt = ap_.tile([P, TCH], f16, name="pt", tag="pt",
                                      bufs=4)
                        if (tt + fc) % 2 == 0:
                            nc.scalar.copy(pt, wop)
                        else:
                            nc.vector.tensor_copy(pt, wop)
                        nc.sync.dma_start(
                            p1g[g][coff + tt * P: coff + (tt + 1) * P,
                                   fc * TCH:(fc + 1) * TCH], pt)
                if (gc + 1) % GPA == 0:
                    nc.gpsimd.collective_compute(
                        "ReduceScatter", mybir.AluOpType.add,
                        replica_groups=rgroups,
                        ins=[p1g[g][:, :]], outs=[a1g[g][:, :]])
                    mlp_prep(g)
        psat.release()
        ap_.release()
        persist.release()

        # ========== local MLP on own 4x128 tokens (full FF, streamed) =======
        mp = tc.alloc_tile_pool(name="mlp", bufs=1)
        psm = tc.alloc_tile_pool(name="psumm", bufs=1, space="PSUM")
        # gate/up
        acs = []
        for m in range(FFT):
            wgt = mp.tile([P, KD * P], f16, name="wgt", tag="wgt", bufs=3)
            nc.sync.dma_start(wgt, wgd[m * P:(m + 1) * P, :])
            wut = mp.tile([P, KD * P], f16, name="wut", tag="wut", bufs=3)
            nc.sync.dma_start(wut, wud[m * P:(m + 1) * P, :])
            gp = psm.tile([P, NAR * P], f32, name="gp", tag="gp", bufs=2)
            for i in range(KD):
                nc.tensor.matmul(gp, wgt[:, i * P:(i + 1) * P], x1nT[i],
                                 start=(i == 0), stop=(i == KD - 1))
            up = psm.tile([P, NAR * P], f32, name="upp", tag="upp", bufs=2)
            for i in range(KD):
                nc.tensor.matmul(up, wut[:, i * P:(i + 1) * P], x1nT[i],
                                 start=(i == 0), stop=(i == KD - 1))
            gs = mp.tile([P, NAR * P], f16, name="gs", tag="gs", bufs=2)
            nc.scalar.activation(gs, gp, mybir.ActivationFunctionType.Silu)
            ac = mp.tile([P, NAR * P], f16, name="ac", tag="ac", bufs=FFT + 1)
            nc.vector.tensor_mul(ac, gs, up)
            acs.append(ac)
        # down + final residual
        for fc in range(FC):
            dps = [psm.tile([P, TCH], f32, name=f"dp{tt}", tag=f"dp{tt}",
                            bufs=1) for tt in range(NAR)]
            for m in range(FFT):
                wdt = mp.tile([P, TCH], f16, name="wdt", tag="wdt", bufs=4)
                nc.sync.dma_start(wdt, wdd[m * P:(m + 1) * P,
                                           fc * TCH:(fc + 1) * TCH])
                for tt in range(NAR):
                    nc.tensor.matmul(dps[tt], acs[m][:, tt * P:(tt + 1) * P],
                                     wdt, start=(m == 0), stop=(m == FFT - 1))
            for tt in range(NAR):
                yt = mp.tile([P, TCH], f32, name="yt", tag="yt", bufs=3)
                nc.vector.tensor_add(yt, dps[tt],
                                     x1g[tt][:, fc * TCH:(fc + 1) * TCH])
                nc.sync.dma_start(
                    yOut[tt * P:(tt + 1) * P, fc * TCH:(fc + 1) * TCH], yt)
        psm.release()
        mp.release()
        mprep.release()

    nc.compile()
    return nc


# ---------------- host side ----------------

_BUILT = {}


def _get_program(cfg_key, cfg):
    if cfg_key not in _BUILT:
        _BUILT[cfg_key] = build_decoder(cfg)
    return _BUILT[cfg_key]


def _host_prep(cfg, x, position_ids, Wq, Wk, Wv, Wo, Wg, Wu, Wd, g1, g2):
    c = _derive(cfg)
    D, N, DH, HD = c["D"], c["N"], c["DH"], c["HD"]
    KD, FFT, NAR, GT = c["KD"], c["FFT"], c["NAR"], c["GT"]
    FF = cfg["FF"]
    xN = np.asarray(x).reshape(N, D).astype(np.float32)
    xN16 = xN.astype(np.float16)
    xT16 = np.ascontiguousarray(xN16.T)

    pos = np.asarray(position_ids).reshape(-1).astype(np.float32)
    inv_freq = (1.0 / (BASE ** (np.arange(0, HD, 2, dtype=np.float32) / HD)))
    ang = pos[:, None] * inv_freq[None, :]           # [N, HD/2]
    cos_f = np.concatenate([np.cos(ang), np.cos(ang)], axis=1)  # [N, HD]
    sin_f = np.concatenate([np.sin(ang), np.sin(ang)], axis=1)
    s = 1.0 / math.sqrt(HD)
    cqt = np.ascontiguousarray(cos_f.T * s).astype(np.float16)
    sqt = np.ascontiguousarray(sin_f.T * s).astype(np.float16)
    ckt = np.ascontiguousarray(cos_f.T).astype(np.float16)
    skt = np.ascontiguousarray(sin_f.T).astype(np.float16)
    # rotate-half as a permutation matrix: rot(q)[d] = sign(d) * q[(d+64) % 128]
    # lhsT layout for the PE: rotm[k, d] = sign(d) * (k == (d+64) % 128)
    rotm = np.zeros((P, P), np.float16)
    for dd in range(P):
        sgn = -1.0 if dd < P // 2 else 1.0
        rotm[(dd + P // 2) % P, dd] = sgn

    # [k, q] score layout: invalid where k > q
    ii, jj = np.indices((P, P))
    maskv = np.where(ii > jj, np.float32(-10000.0), np.float32(0.0))

    g1f = np.asarray(g1, np.float32)[:, None]
    g2f = np.asarray(g2, np.float32)[:, None]
    wqs = (g1f * np.asarray(Wq, np.float32)).astype(np.float16)
    wks = (g1f * np.asarray(Wk, np.float32)).astype(np.float16)
    wvs = (g1f * np.asarray(Wv, np.float32)).astype(np.float16)
    wgs = (g2f * np.asarray(Wg, np.float32)).astype(np.float16)
    wus = (g2f * np.asarray(Wu, np.float32)).astype(np.float16)
    wds = np.asarray(Wd, np.float32).astype(np.float16)
    wos = np.asarray(Wo, np.float32).astype(np.float16)

    # swizzle [D, M]-shaped weights so each 128-wide output tile's [P, KD*P]
    # lhsT block is one contiguous DMA: w_swz[m*P+p, k*P+j] = w[k*P+p, m*P+j]
    def _swz(w):
        mt = w.shape[1] // P
        return np.ascontiguousarray(
            w.reshape(KD, P, mt, P).transpose(2, 1, 0, 3)
            .reshape(mt * P, KD * P))

    wgd = _swz(wgs)
    wud = _swz(wus)

    in_maps = []
    for i in range(NCORES):
        qs = slice(i * DH, (i + 1) * DH)
        xo = np.concatenate(
            [xN16[g * GT + i * P: g * GT + (i + 1) * P] for g in range(NAR)],
            axis=0)
        in_maps.append({
            "xT16": xT16, "xown": np.ascontiguousarray(xo),
            "cq": cqt, "sq": sqt, "ck": ckt, "sk": skt,
            "maskd": maskv, "rotmd": rotm,
            "wqkv": _swz(
                np.concatenate([wqs[:, qs], wks[:, qs], wvs[:, qs]], axis=1)),
            "wo": np.ascontiguousarray(wos[qs, :]),
            "wgd": wgd, "wud": wud, "wdd": wds,
        })
    return in_maps


def run(cfg, inputs, **run_kwargs):
    key = tuple(sorted(cfg.items()))
    nc = _get_program(key, cfg)
    c = _derive(cfg)
    in_maps = _host_prep(cfg, **inputs)
    res = bass_utils.run_bass_kernel_spmd(
        nc, in_maps, core_ids=list(range(NCORES)), **run_kwargs)
    N, D, NAR, GT = c["N"], c["D"], c["NAR"], c["GT"]
    y = np.empty((N, D), np.float32)
    for i in range(NCORES):
        yo = np.asarray(res.results[i]["yOut"])
        for g in range(NAR):
            y[g * GT + i * P: g * GT + (i + 1) * P] = yo[g * P:(g + 1) * P]
    return y.reshape(cfg["B"], cfg["T"], cfg["D"]), res


def kernel(**inputs):
    y, _ = run(FULL_CFG, inputs)
    return y
